# revision 36
# baseline (speedup 1.0000x reference)
"""Dense transformer block (QKV -> causal attention -> out-proj -> FFN+ReLU)
on 8 Trainium2 NeuronCores, data-parallel over the batch dimension.

Contract: kernel(**inputs) takes the FULL inputs
  x [8, 1024, 1024] f32, Wq/Wk/Wv/Wo/W1 [1024, 1024] f32, bo/b1 [1024] f32
and returns the FULL output [8, 1024, 1024] f32.

Each of the 8 cores runs the identical single-core program on one batch
element (batch=8, cores=8 -> no collectives needed).

Single-core design (bf16 tensor-engine compute, fp32 accumulation), v2:
  - The token dim is processed in two strips of 512 queries so that
    early phases overlap late ones: transposes/QKV of strip 0 ->
    attention strip 0 -> QKV strip 1 -> attention strip 1 (with the
    strip-0 output projection and FFN interleaved between its head
    pairs) -> out-proj/FFN strip 1.
  - Scores use true K=64 matmuls placed in PE quadrants via
    tile_position (even head rows 0-63, odd head rows 64-127); both
    heads of a pair write the two banks of one [128, 1024] PSUM tile
    so a single strided ACT instruction exponentiates the pair.
  - Causal masking of the diagonal 128x128 sub-block is done on the
    tensor engine by accumulating a constant -30000 strictly-lower
    triangular tile (identity-stationary matmul) into the score PSUM
    before the exp; fully-future blocks are skipped and partially
    valid blocks only compute their valid column range.
  - V carries an extra all-ones column per head ("augmented V") so the
    softmax denominators fall out of the attn@v matmul as row Dh.
  - Normalization per head pair: DVE copies the two sum rows, a fast
    approximate reciprocal (18-bit) inverts them, a broadcast DMA
    expands each to 64 partitions, and one scalar_tensor_tensor per
    head multiplies the PSUM attention output into bf16 aoutT.
  - out-proj emits projT feature-major (= the lhsT the FFN needs) with
    bo fused via the ACT bias port; FFN emits token-major with b1
    folded in via a K=1 ones-row matmul, ReLU on PSUM eviction.
"""

import numpy as np
from contextlib import ExitStack

import concourse.bass as bass
import concourse.bacc as bacc
import concourse.tile as tile
from concourse import mybir
from concourse.bass_utils import run_bass_kernel_spmd
from concourse.masks import make_identity

F32 = mybir.dt.float32
BF16 = mybir.dt.bfloat16

N_CORES = 8
BATCH = 8
T = 1024
E = 1024
H = 16
DH = 64


def build_nc(TT=T, EE=E, HH=H, Dh=DH):
    nc = bacc.Bacc("TRN2", target_bir_lowering=False, num_swdge_queues=4)

    x = nc.dram_tensor("x", [TT, EE], F32, kind="ExternalInput")
    Wq = nc.dram_tensor("Wq", [EE, EE], F32, kind="ExternalInput")
    Wk = nc.dram_tensor("Wk", [EE, EE], F32, kind="ExternalInput")
    Wv = nc.dram_tensor("Wv", [EE, EE], F32, kind="ExternalInput")
    Wo = nc.dram_tensor("Wo", [EE, EE], F32, kind="ExternalInput")
    bo = nc.dram_tensor("bo", [EE], F32, kind="ExternalInput")
    W1 = nc.dram_tensor("W1", [EE, EE], F32, kind="ExternalInput")
    b1 = nc.dram_tensor("b1", [EE], F32, kind="ExternalInput")
    out = nc.dram_tensor("out", [TT, EE], F32, kind="ExternalOutput")

    EC = EE // 128          # feature-chunk count (partition tiles)
    TC = TT // 128          # token-chunk count
    QT = 512                # query-strip width
    NS = TT // QT           # number of strips (2)
    CS = TC // NS           # token chunks per strip (4)
    QE = 512                # output-feature free-dim chunk for V/FFN
    NE = EE // QE
    HP = 128 // Dh          # heads per 128-partition feature tile (2)
    scale = float(Dh) ** -0.5
    Exp = mybir.ActivationFunctionType.Exp
    Relu = mybir.ActivationFunctionType.Relu
    Ident = mybir.ActivationFunctionType.Identity
    Mult = mybir.AluOpType.mult

    with ExitStack() as ctx:
        tc = ctx.enter_context(tile.TileContext(nc))
        wpool = ctx.enter_context(tc.tile_pool(name="w", bufs=5 * EC))
        xstagep = ctx.enter_context(tc.tile_pool(name="xstage", bufs=2))
        xtokp = ctx.enter_context(tc.tile_pool(name="xtok", bufs=2))
        xTp = ctx.enter_context(tc.tile_pool(name="xT", bufs=EC))
        qTp = ctx.enter_context(tc.tile_pool(name="qT", bufs=EC))
        kTp = ctx.enter_context(tc.tile_pool(name="kT", bufs=EC))
        vp = ctx.enter_context(tc.tile_pool(name="v", bufs=TC))
        ptp = ctx.enter_context(tc.tile_pool(name="pt", bufs=4))
        sumsp = ctx.enter_context(tc.tile_pool(name="sums", bufs=2))
        rbp = ctx.enter_context(tc.tile_pool(name="rb", bufs=2))
        aoutp = ctx.enter_context(tc.tile_pool(name="aout", bufs=EC))
        projp = ctx.enter_context(tc.tile_pool(name="proj", bufs=EC))
        constp = ctx.enter_context(tc.tile_pool(name="const", bufs=1))
        ffoutp = ctx.enter_context(tc.tile_pool(name="ffout", bufs=2))
        ps_acc = ctx.enter_context(tc.tile_pool(name="ps_acc", bufs=2, space="PSUM"))
        ps_s = ctx.enter_context(tc.tile_pool(name="ps_s", bufs=2, space="PSUM"))
        ps_o = ctx.enter_context(tc.tile_pool(name="ps_o", bufs=2, space="PSUM"))

        def load_w(wdram):
            tiles = []
            for ei in range(EC):
                wt = wpool.tile([128, EE], BF16, tag="w")
                nc.gpsimd.dma_start(out=wt, in_=wdram[128 * ei:128 * (ei + 1), :])
                tiles.append(wt)
            return tiles

        # wq first on the gpsimd queue: its arrival gates the first GEMM
        wq = load_w(Wq)

        # ---- constants ----
        bo_sb = constp.tile([128, EC], F32)
        nc.sync.dma_start(out=bo_sb, in_=bo.rearrange("(c p) -> p c", p=128))
        ones_t = constp.tile([1, 128], BF16)
        nc.vector.memset(ones_t, 1.0)
        ident = constp.tile([128, 128], BF16)
        make_identity(nc, ident)
        b1_sb = constp.tile([1, EE], BF16)
        nc.gpsimd.dma_start(out=b1_sb, in_=b1.rearrange("(a e) -> a e", a=1))

        wk = load_w(Wk)
        wv = load_w(Wv)

        # ---- x chunk staging: fp32 DMA -> vector bf16 cast -> PE transpose ----
        xT = [xTp.tile([128, TT], BF16, name="xT", tag="xT") for _ in range(EC)]

        def stage_x(ti):
            xstage = xstagep.tile([128, EE], F32)
            half = EE // 2
            for z, dma_eng in enumerate((nc.sync, nc.scalar)):
                dma_eng.dma_start(
                    out=xstage[:, half * z:half * (z + 1)],
                    in_=x[128 * ti:128 * (ti + 1), half * z:half * (z + 1)],
                )
            xtok = xtokp.tile([128, EE], BF16)
            nc.vector.tensor_copy(out=xtok, in_=xstage)
            return xtok

        def transpose_x(ti, xtok):
            for ec in range(EC):
                if ec % 2 == 0:
                    ps_t = ps_acc.tile([128, 128], BF16, name="ps_t", tag="ps_acc")
                else:
                    ps_t = ps_o.tile([128, 128], BF16, name="ps_t", tag="ops")
                nc.tensor.transpose(
                    ps_t, xtok[:, 128 * ec:128 * (ec + 1)], ident
                )
                dst = xT[ec][:, 128 * ti:128 * (ti + 1)]
                if ec % 2 == 0:
                    nc.vector.tensor_copy(out=dst, in_=ps_t)
                else:
                    nc.scalar.copy(out=dst, in_=ps_t)

        # ---- per-group projection emitters ----
        def proj_group(wtiles, s, eo):
            # one output-feature group of the strip-s projection -> PSUM
            ps = ps_acc.tile([128, QT], F32, name="ps_acc", tag="ps_acc")
            for ei in range(EC):
                nc.tensor.matmul(
                    ps,
                    lhsT=wtiles[ei][:, 128 * eo:128 * (eo + 1)],
                    rhs=xT[ei][:, QT * s:QT * (s + 1)],
                    start=(ei == 0),
                    stop=(ei == EC - 1),
                )
            return ps

        def q_group(s, eo, on_vector=False):
            ps = proj_group(wq, s, eo)
            o = qTp.tile([128, QT], BF16, name="qT", tag="qT")
            if on_vector:
                nc.vector.tensor_copy(out=o, in_=ps)
            else:
                nc.scalar.copy(out=o, in_=ps)
            return o

        def k_group(s, eo, on_vector=False):
            ps = proj_group(wk, s, eo)
            dst = kT[eo][:, QT * s:QT * (s + 1)]
            if on_vector:
                nc.vector.tensor_copy(out=dst, in_=ps)
            else:
                nc.scalar.copy(out=dst, in_=ps)

        kT = [kTp.tile([128, TT], BF16, name="kT", tag="kT") for _ in range(EC)]
        vaug = [vp.tile([128, HH * (Dh + 1)], BF16, name="vaug", tag="vaug")
                for _ in range(TC)]

        def v_group(ti, eoq, on_vector=False):
            va = vaug[ti]
            ps = ps_acc.tile([128, QE], F32, name="ps_acc", tag="ps_acc")
            for ei in range(EC):
                nc.tensor.matmul(
                    ps,
                    lhsT=xT[ei][:, 128 * ti:128 * (ti + 1)],
                    rhs=wv[ei][:, QE * eoq:QE * (eoq + 1)],
                    start=(ei == 0),
                    stop=(ei == EC - 1),
                )
            hq = QE // Dh
            dst = va[:, (Dh + 1) * hq * eoq:(Dh + 1) * hq * (eoq + 1)]
            dst = dst.rearrange("p (h c) -> p h c", c=Dh + 1)[:, :, 0:Dh]
            src = ps.rearrange("p (h d) -> p h d", d=Dh)
            if on_vector:
                nc.vector.tensor_copy(out=dst, in_=src)
            else:
                nc.scalar.copy(out=dst, in_=src)

        aoutT = [aoutp.tile([128, TT], BF16, name="aoutT", tag="aoutT")
                 for _ in range(EC)]

        # ---- attention for one strip; dense_work interleaves PE filler ----
        def attention_strip(s, qTs, dense_work):
            t2max = CS * (s + 1)
            for pi in range(EC):
                opss = [ps_o.tile([Dh + 1, QT], F32, name="ops", tag="ops")
                        for _ in range(HP)]
                pending = []

                def emit_pv(blk):
                    pt, c0, t2 = blk
                    for hi in range(HP):
                        h = HP * pi + hi
                        va_h = vaug[t2][:, h * (Dh + 1):(h + 1) * (Dh + 1)]
                        nc.tensor.matmul(
                            opss[hi][:, c0:QT],
                            lhsT=va_h,
                            rhs=pt[:, QT * hi + c0:QT * (hi + 1)],
                            start=(t2 == 0),
                            stop=(t2 == t2max - 1),
                        )

                for t2 in range(t2max):
                    k0 = 128 * t2 - QT * s
                    c0 = max(0, k0)
                    sp = ps_s.tile([128, HP * QT], F32, name="sp", tag="sp")
                    for hi in range(HP):
                        po = Dh * hi
                        nc.tensor.matmul(
                            sp[:, QT * hi + c0:QT * (hi + 1)],
                            lhsT=kT[pi][po:po + Dh, 128 * t2:128 * (t2 + 1)],
                            rhs=qTs[pi][po:po + Dh, c0:QT],
                            start=True,
                            stop=True,
                        )
                    pt = ptp.tile([128, HP * QT], BF16, name="pt", tag="pt")
                    spv = sp[:, :].rearrange("p (h c) -> p h c", c=QT)[:, :, c0:QT]
                    ptv = pt[:, :].rearrange("p (h c) -> p h c", c=QT)[:, :, c0:QT]
                    nc.scalar.activation(out=ptv, in_=spv, func=Exp, scale=scale)
                    if k0 >= 0:
                        # zero the future half of the diagonal 128-block for
                        # both heads in one strided gpsimd select
                        dv = pt[:, :].rearrange("p (h c) -> p h c", c=QT)
                        dv = dv[:, :, c0:c0 + 128]
                        nc.gpsimd.affine_select(
                            out=dv, in_=dv,
                            compare_op=mybir.AluOpType.is_ge, fill=0.0,
                            base=0, pattern=[[0, HP], [1, 128]],
                            channel_multiplier=-1,
                        )
                    pending.append((pt, c0, t2))
                    if len(pending) > 1:
                        emit_pv(pending.pop(0))
                for blk in pending:
                    emit_pv(blk)

                # ---- fast PSUM eviction: raw out + sum rows; frees ps_o ----
                aslices = []
                sums_t = []
                for hi in range(HP):
                    aslice = aoutT[pi][Dh * hi:Dh * (hi + 1), QT * s:QT * (s + 1)]
                    nc.vector.tensor_copy(out=aslice, in_=opss[hi][0:Dh, :])
                    aslices.append(aslice)
                    sums = sumsp.tile([1, QT], BF16, name="sums", tag="sums")
                    seng = nc.scalar if hi == 0 else nc.vector
                    if hi == 0:
                        seng.copy(out=sums, in_=opss[hi][Dh:Dh + 1, :])
                    else:
                        seng.tensor_copy(out=sums, in_=opss[hi][Dh:Dh + 1, :])
                    sums_t.append(sums)
                # dense filler keeps the PE busy while normalization trails
                if dense_work is not None:
                    dense_work(pi)
                # ---- deferred normalization: rank-1 broadcast on the PE ----
                # rb_raw[64*hi + p, q] = sums_hi[q]; one reciprocal covers both
                rb_raw = ps_o.tile([128, QT], F32, name="rb_raw", tag="ops")
                for hi in range(HP):
                    nc.tensor.matmul(
                        rb_raw[Dh * hi:Dh * (hi + 1), :],
                        lhsT=ones_t[:, 0:Dh],
                        rhs=sums_t[hi],
                        start=True, stop=True,
                    )
                rbs = rbp.tile([128, QT], F32, name="rbs", tag="rbs")
                nc.vector.tensor_copy(out=rbs, in_=rb_raw)
                rb = rbp.tile([128, QT], F32, name="rb", tag="rb")
                nc.vector.reciprocal_approx_fast(out=rb, in_=rbs)
                for hi in range(HP):
                    nc.vector.tensor_mul(
                        out=aslices[hi], in0=aslices[hi],
                        in1=rb[Dh * hi:Dh * (hi + 1), :],
                    )

        # ---- out-projection / FFN emitters (per strip, chunked) ----
        def outproj_eo(wo, s, eo):
            ps = ps_acc.tile([128, QT], F32, name="ps_acc", tag="ps_acc")
            for ei in range(EC):
                nc.tensor.matmul(
                    ps,
                    lhsT=wo[ei][:, 128 * eo:128 * (eo + 1)],
                    rhs=aoutT[ei][:, QT * s:QT * (s + 1)],
                    start=(ei == 0),
                    stop=(ei == EC - 1),
                )
            nc.scalar.activation(
                out=projT[eo][:, QT * s:QT * (s + 1)], in_=ps,
                func=Ident, bias=bo_sb[:, eo:eo + 1], scale=1.0,
            )

        def ffn_group(w1, s, g):
            # g in [0, 2*CS): token chunk = s*CS + g//NE, eoq = g%NE
            ti = CS * s + g // NE
            eoq = g % NE
            ps = ps_acc.tile([128, QE], F32, name="ps_acc", tag="ps_acc")
            for ei in range(EC):
                nc.tensor.matmul(
                    ps,
                    lhsT=projT[ei][:, 128 * ti:128 * (ti + 1)],
                    rhs=w1[ei][:, QE * eoq:QE * (eoq + 1)],
                    start=(ei == 0),
                    stop=False,
                )
            nc.tensor.matmul(
                ps,
                lhsT=ones_t[:, 0:128],
                rhs=b1_sb[:, QE * eoq:QE * (eoq + 1)],
                start=False,
                stop=True,
            )
            fo = ffoutp.tile([128, QE], F32)
            nc.scalar.activation(out=fo, in_=ps, func=Relu)
            # split the store across two DMA queues, rotating per group
            half = QE // 2
            qs = [(nc.sync, nc.gpsimd), (nc.gpsimd, nc.sync)][g % 2]
            for z, qeng in enumerate(qs):
                qeng.dma_start(
                    out=out[128 * ti:128 * (ti + 1),
                            QE * eoq + half * z:QE * eoq + half * (z + 1)],
                    in_=fo[:, half * z:half * (z + 1)],
                )

        # ================= program order =================
        # strip 0 inputs
        xtoks = {}
        for ti in range(TC):
            xtoks[ti] = stage_x(ti)
            if ti < CS:
                transpose_x(ti, xtoks[ti])
        for ti in range(CS):
            nc.gpsimd.memset(vaug[ti], 1.0)
        qT0 = [q_group(0, eo) for eo in range(EC)]
        for eo in range(EC):
            k_group(0, eo)
        for ti in range(CS):
            for eoq in range(NE):
                v_group(ti, eoq)
        wo = load_w(Wo)
        w1 = load_w(W1)
        for ti in range(CS, TC):
            nc.gpsimd.memset(vaug[ti], 1.0)
        for ti in range(CS, TC):
            transpose_x(ti, xtoks[ti])
        projT = [projp.tile([128, TT], BF16, name="projT", tag="projT")
                 for _ in range(EC)]

        # attention strip 0 with strip-1 Q/K/V interleaved between pairs
        qT1 = [None] * EC

        def qkv1_filler(pi):
            qT1[pi] = q_group(1, pi, on_vector=True)
            k_group(1, pi, on_vector=True)
            v_group(CS + pi // NE, pi % NE, on_vector=True)

        attention_strip(0, qT0, qkv1_filler)

        # attention strip 1 with strip-0 out-proj/FFN interleaved
        def dense_filler(pi):
            if pi < 4:
                outproj_eo(wo, 0, 2 * pi)
                outproj_eo(wo, 0, 2 * pi + 1)
            else:
                ffn_group(w1, 0, 2 * (pi - 4))
                ffn_group(w1, 0, 2 * (pi - 4) + 1)

        attention_strip(1, qT1, dense_filler)

        for eo in range(EC):
            outproj_eo(wo, 1, eo)
        for g in range(2 * CS):
            ffn_group(w1, 1, g)

    nc.finalize()
    return nc


_NC_CACHE = {}


def _get_nc(shape_key):
    if shape_key not in _NC_CACHE:
        _NC_CACHE[shape_key] = build_nc(*shape_key)
    return _NC_CACHE[shape_key]


def kernel(x, Wq, Wk, Wv, Wo, bo, W1, b1):
    x = np.ascontiguousarray(np.asarray(x, dtype=np.float32))
    ws = {
        "Wq": np.ascontiguousarray(np.asarray(Wq, dtype=np.float32)),
        "Wk": np.ascontiguousarray(np.asarray(Wk, dtype=np.float32)),
        "Wv": np.ascontiguousarray(np.asarray(Wv, dtype=np.float32)),
        "Wo": np.ascontiguousarray(np.asarray(Wo, dtype=np.float32)),
        "bo": np.ascontiguousarray(np.asarray(bo, dtype=np.float32)),
        "W1": np.ascontiguousarray(np.asarray(W1, dtype=np.float32)),
        "b1": np.ascontiguousarray(np.asarray(b1, dtype=np.float32)),
    }
    B, TT, EE = x.shape
    assert B == N_CORES
    nc = _get_nc((TT, EE, H, DH))
    in_maps = [dict(ws, x=x[b]) for b in range(B)]
    res = run_bass_kernel_spmd(nc, in_maps, core_ids=list(range(N_CORES)))
    return np.stack([res.results[b]["out"] for b in range(B)], axis=0).astype(
        np.float32
    )


# revision 40
# speedup vs baseline: 1.1892x; 1.1892x over previous
"""Dense transformer block (QKV -> causal attention -> out-proj -> FFN+ReLU)
on 8 Trainium2 NeuronCores, data-parallel over the batch dimension.

Contract: kernel(**inputs) takes the FULL inputs
  x [8, 1024, 1024] f32, Wq/Wk/Wv/Wo/W1 [1024, 1024] f32, bo/b1 [1024] f32
and returns the FULL output [8, 1024, 1024] f32.

Each of the 8 cores runs the identical single-core program on one batch
element (batch=8, cores=8 -> no collectives needed).

Single-core design (bf16 tensor-engine compute, fp32 accumulation), v2:
  - The token dim is processed in two strips of 512 queries so that
    early phases overlap late ones: transposes/QKV of strip 0 ->
    attention strip 0 -> QKV strip 1 -> attention strip 1 (with the
    strip-0 output projection and FFN interleaved between its head
    pairs) -> out-proj/FFN strip 1.
  - Scores use true K=64 matmuls placed in PE quadrants via
    tile_position (even head rows 0-63, odd head rows 64-127); both
    heads of a pair write the two banks of one [128, 1024] PSUM tile
    so a single strided ACT instruction exponentiates the pair.
  - Causal masking of the diagonal 128x128 sub-block is done on the
    tensor engine by accumulating a constant -30000 strictly-lower
    triangular tile (identity-stationary matmul) into the score PSUM
    before the exp; fully-future blocks are skipped and partially
    valid blocks only compute their valid column range.
  - V carries an extra all-ones column per head ("augmented V") so the
    softmax denominators fall out of the attn@v matmul as row Dh.
  - Normalization per head pair: DVE copies the two sum rows, a fast
    approximate reciprocal (18-bit) inverts them, a broadcast DMA
    expands each to 64 partitions, and one scalar_tensor_tensor per
    head multiplies the PSUM attention output into bf16 aoutT.
  - out-proj emits projT feature-major (= the lhsT the FFN needs) with
    bo fused via the ACT bias port; FFN emits token-major with b1
    folded in via a K=1 ones-row matmul, ReLU on PSUM eviction.
"""

import numpy as np
from contextlib import ExitStack

import concourse.bass as bass
import concourse.bacc as bacc
import concourse.tile as tile
from concourse import mybir
from concourse.bass_utils import run_bass_kernel_spmd
from concourse.masks import make_identity

F32 = mybir.dt.float32
BF16 = mybir.dt.bfloat16

N_CORES = 8
BATCH = 8
T = 1024
E = 1024
H = 16
DH = 64


def build_nc(TT=T, EE=E, HH=H, Dh=DH):
    nc = bacc.Bacc("TRN2", target_bir_lowering=False, num_swdge_queues=4)

    x = nc.dram_tensor("x", [TT, EE], F32, kind="ExternalInput")
    Wq = nc.dram_tensor("Wq", [EE, EE], F32, kind="ExternalInput")
    Wk = nc.dram_tensor("Wk", [EE, EE], F32, kind="ExternalInput")
    Wv = nc.dram_tensor("Wv", [EE, EE], F32, kind="ExternalInput")
    Wo = nc.dram_tensor("Wo", [EE, EE], F32, kind="ExternalInput")
    bo = nc.dram_tensor("bo", [EE], F32, kind="ExternalInput")
    W1 = nc.dram_tensor("W1", [EE, EE], F32, kind="ExternalInput")
    b1 = nc.dram_tensor("b1", [EE], F32, kind="ExternalInput")
    out = nc.dram_tensor("out", [TT, EE], F32, kind="ExternalOutput")

    EC = EE // 128          # feature-chunk count (partition tiles)
    TC = TT // 128          # token-chunk count
    QT = 512                # query-strip width
    NS = TT // QT           # number of strips (2)
    CS = TC // NS           # token chunks per strip (4)
    QE = 512                # output-feature free-dim chunk for V/FFN
    NE = EE // QE
    HP = 128 // Dh          # heads per 128-partition feature tile (2)
    scale = float(Dh) ** -0.5
    Exp = mybir.ActivationFunctionType.Exp
    Relu = mybir.ActivationFunctionType.Relu
    Ident = mybir.ActivationFunctionType.Identity
    Mult = mybir.AluOpType.mult

    with ExitStack() as ctx:
        tc = ctx.enter_context(tile.TileContext(nc))
        wpool = ctx.enter_context(tc.tile_pool(name="w", bufs=5 * EC))
        xstagep = ctx.enter_context(tc.tile_pool(name="xstage", bufs=2))
        xtokp = ctx.enter_context(tc.tile_pool(name="xtok", bufs=2))
        xTp = ctx.enter_context(tc.tile_pool(name="xT", bufs=EC))
        qTp = ctx.enter_context(tc.tile_pool(name="qT", bufs=EC))
        kTp = ctx.enter_context(tc.tile_pool(name="kT", bufs=EC))
        vp = ctx.enter_context(tc.tile_pool(name="v", bufs=TC))
        ptp = ctx.enter_context(tc.tile_pool(name="pt", bufs=4))
        sumsp = ctx.enter_context(tc.tile_pool(name="sums", bufs=2))
        rbp = ctx.enter_context(tc.tile_pool(name="rb", bufs=2))
        aoutp = ctx.enter_context(tc.tile_pool(name="aout", bufs=EC))
        projp = ctx.enter_context(tc.tile_pool(name="proj", bufs=EC))
        constp = ctx.enter_context(tc.tile_pool(name="const", bufs=1))
        ffoutp = ctx.enter_context(tc.tile_pool(name="ffout", bufs=2))
        ps_acc = ctx.enter_context(tc.tile_pool(name="ps_acc", bufs=2, space="PSUM"))
        ps_s = ctx.enter_context(tc.tile_pool(name="ps_s", bufs=2, space="PSUM"))
        ps_o = ctx.enter_context(tc.tile_pool(name="ps_o", bufs=2, space="PSUM"))

        def load_w(wdram):
            tiles = []
            for ei in range(EC):
                wt = wpool.tile([128, EE], BF16, tag="w")
                nc.gpsimd.dma_start(out=wt, in_=wdram[128 * ei:128 * (ei + 1), :])
                tiles.append(wt)
            return tiles

        # wq first on the gpsimd queue: its arrival gates the first GEMM
        wq = load_w(Wq)

        # ---- constants ----
        bo_sb = constp.tile([128, EC], F32)
        nc.sync.dma_start(out=bo_sb, in_=bo.rearrange("(c p) -> p c", p=128))
        ones_t = constp.tile([1, 128], BF16)
        nc.vector.memset(ones_t, 1.0)
        ident = constp.tile([128, 128], BF16)
        make_identity(nc, ident)
        b1_sb = constp.tile([1, EE], BF16)
        nc.gpsimd.dma_start(out=b1_sb, in_=b1.rearrange("(a e) -> a e", a=1))

        wk = load_w(Wk)
        wv = load_w(Wv)

        # ---- x chunk staging: fp32 DMA -> vector bf16 cast -> PE transpose ----
        xT = [xTp.tile([128, TT], BF16, name="xT", tag="xT") for _ in range(EC)]

        def stage_x(ti):
            xstage = xstagep.tile([128, EE], F32)
            half = EE // 2
            for z, dma_eng in enumerate((nc.sync, nc.scalar)):
                dma_eng.dma_start(
                    out=xstage[:, half * z:half * (z + 1)],
                    in_=x[128 * ti:128 * (ti + 1), half * z:half * (z + 1)],
                )
            xtok = xtokp.tile([128, EE], BF16)
            nc.vector.tensor_copy(out=xtok, in_=xstage)
            return xtok

        def transpose_x(ti, xtok):
            for ec in range(EC):
                if ec % 2 == 0:
                    ps_t = ps_acc.tile([128, 128], BF16, name="ps_t", tag="ps_acc")
                else:
                    ps_t = ps_o.tile([128, 128], BF16, name="ps_t", tag="ops")
                nc.tensor.transpose(
                    ps_t, xtok[:, 128 * ec:128 * (ec + 1)], ident
                )
                dst = xT[ec][:, 128 * ti:128 * (ti + 1)]
                if ec % 2 == 0:
                    nc.vector.tensor_copy(out=dst, in_=ps_t)
                else:
                    nc.scalar.copy(out=dst, in_=ps_t)

        # ---- weight-arrival-paced projection (first GEMMs only) ----
        # ei-outer over four output groups at once: matmuls start as soon
        # as the first weight tiles land instead of waiting for all eight.
        def proj_half_paced(wtiles, s, eo_base, evict):
            pss = [ps_acc.tile([128, QT], F32, name="ps_acc", tag="ps_acc")
                   for _ in range(2)]
            sp = ps_s.tile([128, HP * QT], F32, name="sp", tag="sp")
            psums = pss + [sp[:, 0:QT], sp[:, QT:2 * QT]]
            for ei in range(EC):
                for j in range(4):
                    eo = eo_base + j
                    nc.tensor.matmul(
                        psums[j],
                        lhsT=wtiles[ei][:, 128 * eo:128 * (eo + 1)],
                        rhs=xT[ei][:, QT * s:QT * (s + 1)],
                        start=(ei == 0),
                        stop=(ei == EC - 1),
                    )
            for j in range(4):
                evict(eo_base + j, psums[j])

        # ---- per-group projection emitters ----
        def proj_group(wtiles, s, eo):
            # one output-feature group of the strip-s projection -> PSUM
            ps = ps_acc.tile([128, QT], F32, name="ps_acc", tag="ps_acc")
            for ei in range(EC):
                nc.tensor.matmul(
                    ps,
                    lhsT=wtiles[ei][:, 128 * eo:128 * (eo + 1)],
                    rhs=xT[ei][:, QT * s:QT * (s + 1)],
                    start=(ei == 0),
                    stop=(ei == EC - 1),
                )
            return ps

        def q_group(s, eo, on_vector=False):
            ps = proj_group(wq, s, eo)
            o = qTp.tile([128, QT], BF16, name="qT", tag="qT")
            if on_vector:
                nc.vector.tensor_copy(out=o, in_=ps)
            else:
                nc.scalar.copy(out=o, in_=ps)
            return o

        def k_group(s, eo, on_vector=False):
            ps = proj_group(wk, s, eo)
            dst = kT[eo][:, QT * s:QT * (s + 1)]
            if on_vector:
                nc.vector.tensor_copy(out=dst, in_=ps)
            else:
                nc.scalar.copy(out=dst, in_=ps)

        kT = [kTp.tile([128, TT], BF16, name="kT", tag="kT") for _ in range(EC)]
        vaug = [vp.tile([128, HH * (Dh + 1)], BF16, name="vaug", tag="vaug")
                for _ in range(TC)]

        def v_group(ti, eoq, on_vector=False):
            va = vaug[ti]
            ps = ps_acc.tile([128, QE], F32, name="ps_acc", tag="ps_acc")
            for ei in range(EC):
                nc.tensor.matmul(
                    ps,
                    lhsT=xT[ei][:, 128 * ti:128 * (ti + 1)],
                    rhs=wv[ei][:, QE * eoq:QE * (eoq + 1)],
                    start=(ei == 0),
                    stop=(ei == EC - 1),
                )
            hq = QE // Dh
            dst = va[:, (Dh + 1) * hq * eoq:(Dh + 1) * hq * (eoq + 1)]
            dst = dst.rearrange("p (h c) -> p h c", c=Dh + 1)[:, :, 0:Dh]
            src = ps.rearrange("p (h d) -> p h d", d=Dh)
            if on_vector:
                nc.vector.tensor_copy(out=dst, in_=src)
            else:
                nc.scalar.copy(out=dst, in_=src)

        aoutT = [aoutp.tile([128, TT], BF16, name="aoutT", tag="aoutT")
                 for _ in range(EC)]

        # ---- attention for one strip; dense_work interleaves PE filler ----
        def attention_strip(s, qTs, dense_work):
            t2max = CS * (s + 1)
            for pi in range(EC):
                opss = [ps_o.tile([Dh + 1, QT], F32, name="ops", tag="ops")
                        for _ in range(HP)]
                pending = []

                def emit_pv(blk):
                    pt, c0, t2 = blk
                    for hi in range(HP):
                        h = HP * pi + hi
                        va_h = vaug[t2][:, h * (Dh + 1):(h + 1) * (Dh + 1)]
                        nc.tensor.matmul(
                            opss[hi][:, c0:QT],
                            lhsT=va_h,
                            rhs=pt[:, QT * hi + c0:QT * (hi + 1)],
                            start=(t2 == 0),
                            stop=(t2 == t2max - 1),
                        )

                for t2 in range(t2max):
                    k0 = 128 * t2 - QT * s
                    c0 = max(0, k0)
                    sp = ps_s.tile([128, HP * QT], F32, name="sp", tag="sp")
                    for hi in range(HP):
                        po = Dh * hi
                        nc.tensor.matmul(
                            sp[:, QT * hi + c0:QT * (hi + 1)],
                            lhsT=kT[pi][po:po + Dh, 128 * t2:128 * (t2 + 1)],
                            rhs=qTs[pi][po:po + Dh, c0:QT],
                            start=True,
                            stop=True,
                        )
                    pt = ptp.tile([128, HP * QT], BF16, name="pt", tag="pt")
                    spv = sp[:, :].rearrange("p (h c) -> p h c", c=QT)[:, :, c0:QT]
                    ptv = pt[:, :].rearrange("p (h c) -> p h c", c=QT)[:, :, c0:QT]
                    nc.scalar.activation(out=ptv, in_=spv, func=Exp, scale=scale)
                    if k0 >= 0:
                        # zero the future half of the diagonal 128-block for
                        # both heads in one strided gpsimd select
                        dv = pt[:, :].rearrange("p (h c) -> p h c", c=QT)
                        dv = dv[:, :, c0:c0 + 128]
                        nc.gpsimd.affine_select(
                            out=dv, in_=dv,
                            compare_op=mybir.AluOpType.is_ge, fill=0.0,
                            base=0, pattern=[[0, HP], [1, 128]],
                            channel_multiplier=-1,
                        )
                    pending.append((pt, c0, t2))
                    if len(pending) > 1:
                        emit_pv(pending.pop(0))
                for blk in pending:
                    emit_pv(blk)

                # ---- fast PSUM eviction: raw out + sum rows; frees ps_o ----
                aslices = []
                sums_t = []
                for hi in range(HP):
                    aslice = aoutT[pi][Dh * hi:Dh * (hi + 1), QT * s:QT * (s + 1)]
                    nc.vector.tensor_copy(out=aslice, in_=opss[hi][0:Dh, :])
                    aslices.append(aslice)
                    sums = sumsp.tile([1, QT], BF16, name="sums", tag="sums")
                    seng = nc.scalar if hi == 0 else nc.vector
                    if hi == 0:
                        seng.copy(out=sums, in_=opss[hi][Dh:Dh + 1, :])
                    else:
                        seng.tensor_copy(out=sums, in_=opss[hi][Dh:Dh + 1, :])
                    sums_t.append(sums)
                # dense filler keeps the PE busy while normalization trails
                if dense_work is not None:
                    dense_work(pi)
                # ---- deferred normalization: rank-1 broadcast on the PE ----
                # rb_raw[64*hi + p, q] = sums_hi[q]; one reciprocal covers both
                rb_raw = ps_o.tile([128, QT], F32, name="rb_raw", tag="ops")
                for hi in range(HP):
                    nc.tensor.matmul(
                        rb_raw[Dh * hi:Dh * (hi + 1), :],
                        lhsT=ones_t[:, 0:Dh],
                        rhs=sums_t[hi],
                        start=True, stop=True,
                    )
                rbs = rbp.tile([128, QT], F32, name="rbs", tag="rbs")
                nc.vector.tensor_copy(out=rbs, in_=rb_raw)
                rb = rbp.tile([128, QT], F32, name="rb", tag="rb")
                nc.vector.reciprocal_approx_fast(out=rb, in_=rbs)
                for hi in range(HP):
                    nc.vector.tensor_mul(
                        out=aslices[hi], in0=aslices[hi],
                        in1=rb[Dh * hi:Dh * (hi + 1), :],
                    )

        # ---- out-projection / FFN emitters (per strip, chunked) ----
        def outproj_eo(wo, s, eo):
            ps = ps_acc.tile([128, QT], F32, name="ps_acc", tag="ps_acc")
            for ei in range(EC):
                nc.tensor.matmul(
                    ps,
                    lhsT=wo[ei][:, 128 * eo:128 * (eo + 1)],
                    rhs=aoutT[ei][:, QT * s:QT * (s + 1)],
                    start=(ei == 0),
                    stop=(ei == EC - 1),
                )
            nc.scalar.activation(
                out=projT[eo][:, QT * s:QT * (s + 1)], in_=ps,
                func=Ident, bias=bo_sb[:, eo:eo + 1], scale=1.0,
            )

        def ffn_group(w1, s, g):
            # g in [0, 2*CS): token chunk = s*CS + g//NE, eoq = g%NE
            ti = CS * s + g // NE
            eoq = g % NE
            ps = ps_acc.tile([128, QE], F32, name="ps_acc", tag="ps_acc")
            for ei in range(EC):
                nc.tensor.matmul(
                    ps,
                    lhsT=projT[ei][:, 128 * ti:128 * (ti + 1)],
                    rhs=w1[ei][:, QE * eoq:QE * (eoq + 1)],
                    start=(ei == 0),
                    stop=False,
                )
            nc.tensor.matmul(
                ps,
                lhsT=ones_t[:, 0:128],
                rhs=b1_sb[:, QE * eoq:QE * (eoq + 1)],
                start=False,
                stop=True,
            )
            fo = ffoutp.tile([128, QE], F32)
            nc.scalar.activation(out=fo, in_=ps, func=Relu)
            # split the store across two DMA queues, rotating per group
            half = QE // 2
            qs = [(nc.sync, nc.gpsimd), (nc.gpsimd, nc.sync)][g % 2]
            for z, qeng in enumerate(qs):
                qeng.dma_start(
                    out=out[128 * ti:128 * (ti + 1),
                            QE * eoq + half * z:QE * eoq + half * (z + 1)],
                    in_=fo[:, half * z:half * (z + 1)],
                )

        # ================= program order =================
        # strip 0 inputs
        xtoks = {}
        for ti in range(TC):
            xtoks[ti] = stage_x(ti)
            if ti < CS:
                transpose_x(ti, xtoks[ti])
        for ti in range(CS):
            nc.gpsimd.memset(vaug[ti], 1.0)

        qT0 = [None] * EC

        def q_evict(eo, ps):
            o = qTp.tile([128, QT], BF16, name="qT", tag="qT")
            nc.scalar.copy(out=o, in_=ps)
            qT0[eo] = o

        def k_evict(eo, ps):
            nc.scalar.copy(out=kT[eo][:, 0:QT], in_=ps)

        proj_half_paced(wq, 0, 0, q_evict)
        proj_half_paced(wq, 0, 4, q_evict)
        proj_half_paced(wk, 0, 0, k_evict)
        proj_half_paced(wk, 0, 4, k_evict)
        for ti in range(CS):
            for eoq in range(NE):
                v_group(ti, eoq)
        wo = load_w(Wo)
        w1 = load_w(W1)
        for ti in range(CS, TC):
            nc.gpsimd.memset(vaug[ti], 1.0)
        for ti in range(CS, TC):
            transpose_x(ti, xtoks[ti])
        projT = [projp.tile([128, TT], BF16, name="projT", tag="projT")
                 for _ in range(EC)]

        # attention strip 0 with strip-1 Q/K/V interleaved between pairs
        qT1 = [None] * EC

        def qkv1_filler(pi):
            qT1[pi] = q_group(1, pi, on_vector=True)
            k_group(1, pi, on_vector=True)
            v_group(CS + pi // NE, pi % NE, on_vector=True)

        attention_strip(0, qT0, qkv1_filler)

        # attention strip 1 with strip-0 out-proj/FFN interleaved
        def dense_filler(pi):
            if pi < 4:
                outproj_eo(wo, 0, 2 * pi)
                outproj_eo(wo, 0, 2 * pi + 1)
            else:
                ffn_group(w1, 0, 2 * (pi - 4))
                ffn_group(w1, 0, 2 * (pi - 4) + 1)

        attention_strip(1, qT1, dense_filler)

        for eo in range(EC):
            outproj_eo(wo, 1, eo)
        for g in range(2 * CS):
            ffn_group(w1, 1, g)

    nc.finalize()
    return nc


_NC_CACHE = {}


def _get_nc(shape_key):
    if shape_key not in _NC_CACHE:
        _NC_CACHE[shape_key] = build_nc(*shape_key)
    return _NC_CACHE[shape_key]


def kernel(x, Wq, Wk, Wv, Wo, bo, W1, b1):
    x = np.ascontiguousarray(np.asarray(x, dtype=np.float32))
    ws = {
        "Wq": np.ascontiguousarray(np.asarray(Wq, dtype=np.float32)),
        "Wk": np.ascontiguousarray(np.asarray(Wk, dtype=np.float32)),
        "Wv": np.ascontiguousarray(np.asarray(Wv, dtype=np.float32)),
        "Wo": np.ascontiguousarray(np.asarray(Wo, dtype=np.float32)),
        "bo": np.ascontiguousarray(np.asarray(bo, dtype=np.float32)),
        "W1": np.ascontiguousarray(np.asarray(W1, dtype=np.float32)),
        "b1": np.ascontiguousarray(np.asarray(b1, dtype=np.float32)),
    }
    B, TT, EE = x.shape
    assert B == N_CORES
    nc = _get_nc((TT, EE, H, DH))
    in_maps = [dict(ws, x=x[b]) for b in range(B)]
    res = run_bass_kernel_spmd(nc, in_maps, core_ids=list(range(N_CORES)))
    return np.stack([res.results[b]["out"] for b in range(B)], axis=0).astype(
        np.float32
    )


# revision 45
# speedup vs baseline: 1.2045x; 1.0128x over previous
"""Dense transformer block (QKV -> causal attention -> out-proj -> FFN+ReLU)
on 8 Trainium2 NeuronCores, data-parallel over the batch dimension.

Contract: kernel(**inputs) takes the FULL inputs
  x [8, 1024, 1024] f32, Wq/Wk/Wv/Wo/W1 [1024, 1024] f32, bo/b1 [1024] f32
and returns the FULL output [8, 1024, 1024] f32.

Each of the 8 cores runs the identical single-core program on one batch
element (batch=8, cores=8 -> no collectives needed).

Single-core design (bf16 tensor-engine compute, fp32 accumulation), v3:
  - The token dim is processed in two strips of 512 queries so that
    early phases overlap late ones: transposes/QKV of strip 0 ->
    attention strip 0 (with the strip-1 Q/K/V projections interleaved
    between its head pairs) -> attention strip 1 (with the strip-0
    output projection and FFN interleaved) -> out-proj/FFN strip 1.
  - The first two GEMMs (Q0/K0) run ei-outer over four output groups
    at once, paced by the weight-tile DMA arrivals, so matmuls start
    before the full weight matrix has landed.
  - Scores use true K=64 matmuls placed in PE quadrants via
    tile_position (even head rows 0-63, odd head rows 64-127); both
    heads of a pair write the two banks of one [128, 1024] PSUM tile
    so a single strided ACT instruction exponentiates the pair.
  - Causal masking of the diagonal 128x128 sub-block: one strided
    gpsimd affine_select per block zeroes the future half of the exp
    output for both heads; fully-future blocks are skipped and
    partially valid blocks only compute their valid column range.
  - V carries an extra all-ones column per head ("augmented V") so the
    softmax denominators fall out of the attn@v matmul as row Dh.
  - Normalization per head pair: raw PSUM eviction (bf16 attention out
    + sum rows) frees PSUM fast; the 64-partition broadcast of the
    sums is a rank-1 ones(x)sums matmul on the PE, inverted by one
    18-bit reciprocal_approx_fast, then multiplied in place on DVE.
  - out-proj emits projT feature-major (= the lhsT the FFN needs) with
    bo fused via the ACT bias port; FFN emits token-major with b1
    folded in via a K=1 ones-row matmul, ReLU on PSUM eviction, and
    stores split across rotating DMA queues.
"""

import numpy as np
from contextlib import ExitStack

import concourse.bass as bass
import concourse.bacc as bacc
import concourse.tile as tile
from concourse import mybir
from concourse.bass_utils import run_bass_kernel_spmd
from concourse.masks import make_identity

F32 = mybir.dt.float32
BF16 = mybir.dt.bfloat16

N_CORES = 8
BATCH = 8
T = 1024
E = 1024
H = 16
DH = 64


def build_nc(TT=T, EE=E, HH=H, Dh=DH):
    nc = bacc.Bacc("TRN2", target_bir_lowering=False, num_swdge_queues=4)

    x = nc.dram_tensor("x", [TT, EE], F32, kind="ExternalInput")
    Wq = nc.dram_tensor("Wq", [EE, EE], F32, kind="ExternalInput")
    Wk = nc.dram_tensor("Wk", [EE, EE], F32, kind="ExternalInput")
    Wv = nc.dram_tensor("Wv", [EE, EE], F32, kind="ExternalInput")
    Wo = nc.dram_tensor("Wo", [EE, EE], F32, kind="ExternalInput")
    bo = nc.dram_tensor("bo", [EE], F32, kind="ExternalInput")
    W1 = nc.dram_tensor("W1", [EE, EE], F32, kind="ExternalInput")
    b1 = nc.dram_tensor("b1", [EE], F32, kind="ExternalInput")
    out = nc.dram_tensor("out", [TT, EE], F32, kind="ExternalOutput")

    EC = EE // 128          # feature-chunk count (partition tiles)
    TC = TT // 128          # token-chunk count
    QT = 512                # query-strip width
    NS = TT // QT           # number of strips (2)
    CS = TC // NS           # token chunks per strip (4)
    QE = 512                # output-feature free-dim chunk for V/FFN
    NE = EE // QE
    HP = 128 // Dh          # heads per 128-partition feature tile (2)
    scale = float(Dh) ** -0.5
    Exp = mybir.ActivationFunctionType.Exp
    Relu = mybir.ActivationFunctionType.Relu
    Ident = mybir.ActivationFunctionType.Identity
    Mult = mybir.AluOpType.mult

    with ExitStack() as ctx:
        tc = ctx.enter_context(tile.TileContext(nc))
        wpool = ctx.enter_context(tc.tile_pool(name="w", bufs=5 * EC))
        xstagep = ctx.enter_context(tc.tile_pool(name="xstage", bufs=2))
        xtokp = ctx.enter_context(tc.tile_pool(name="xtok", bufs=2))
        xTp = ctx.enter_context(tc.tile_pool(name="xT", bufs=EC))
        qTp = ctx.enter_context(tc.tile_pool(name="qT", bufs=EC))
        kTp = ctx.enter_context(tc.tile_pool(name="kT", bufs=EC))
        vp = ctx.enter_context(tc.tile_pool(name="v", bufs=TC))
        ptp = ctx.enter_context(tc.tile_pool(name="pt", bufs=4))
        sumsp = ctx.enter_context(tc.tile_pool(name="sums", bufs=2))
        rbp = ctx.enter_context(tc.tile_pool(name="rb", bufs=2))
        aoutp = ctx.enter_context(tc.tile_pool(name="aout", bufs=EC))
        projp = ctx.enter_context(tc.tile_pool(name="proj", bufs=EC))
        constp = ctx.enter_context(tc.tile_pool(name="const", bufs=1))
        ffoutp = ctx.enter_context(tc.tile_pool(name="ffout", bufs=2))
        ps_acc = ctx.enter_context(tc.tile_pool(name="ps_acc", bufs=2, space="PSUM"))
        ps_s = ctx.enter_context(tc.tile_pool(name="ps_s", bufs=2, space="PSUM"))
        ps_o = ctx.enter_context(tc.tile_pool(name="ps_o", bufs=2, space="PSUM"))

        def load_w(wdram):
            tiles = []
            for ei in range(EC):
                wt = wpool.tile([128, EE], BF16, tag="w")
                nc.gpsimd.dma_start(out=wt, in_=wdram[128 * ei:128 * (ei + 1), :])
                tiles.append(wt)
            return tiles

        # wq first on the gpsimd queue: its arrival gates the first GEMM
        wq = load_w(Wq)

        # ---- constants ----
        bo_sb = constp.tile([128, EC], F32)
        nc.sync.dma_start(out=bo_sb, in_=bo.rearrange("(c p) -> p c", p=128))
        ones_t = constp.tile([1, 128], BF16)
        nc.vector.memset(ones_t, 1.0)
        ident = constp.tile([128, 128], BF16)
        make_identity(nc, ident)
        b1_sb = constp.tile([1, EE], BF16)
        nc.gpsimd.dma_start(out=b1_sb, in_=b1.rearrange("(a e) -> a e", a=1))

        wk = load_w(Wk)
        wv = load_w(Wv)

        # ---- x chunk staging: fp32 DMA -> vector bf16 cast -> PE transpose ----
        xT = [xTp.tile([128, TT], BF16, name="xT", tag="xT") for _ in range(EC)]

        def stage_x(ti):
            xstage = xstagep.tile([128, EE], F32)
            # split by rows (not columns): each half is a fully contiguous
            # DRAM read, so the DMA moves 4KB bursts instead of strided 2KB
            for z, dma_eng in enumerate((nc.sync, nc.scalar)):
                dma_eng.dma_start(
                    out=xstage[64 * z:64 * (z + 1), :],
                    in_=x[128 * ti + 64 * z:128 * ti + 64 * (z + 1), :],
                )
            xtok = xtokp.tile([128, EE], BF16)
            nc.vector.tensor_copy(out=xtok, in_=xstage)
            return xtok

        def transpose_x(ti, xtok):
            for ec in range(EC):
                if ec % 2 == 0:
                    ps_t = ps_acc.tile([128, 128], BF16, name="ps_t", tag="ps_acc")
                else:
                    ps_t = ps_o.tile([128, 128], BF16, name="ps_t", tag="ops")
                nc.tensor.transpose(
                    ps_t, xtok[:, 128 * ec:128 * (ec + 1)], ident
                )
                dst = xT[ec][:, 128 * ti:128 * (ti + 1)]
                if ec % 2 == 0:
                    nc.vector.tensor_copy(out=dst, in_=ps_t)
                else:
                    nc.scalar.copy(out=dst, in_=ps_t)

        # ---- weight-arrival-paced projection (first GEMMs only) ----
        # ei-outer over four output groups at once: matmuls start as soon
        # as the first weight tiles land instead of waiting for all eight.
        def proj_half_paced(wtiles, s, eo_base, evict):
            pss = [ps_acc.tile([128, QT], F32, name="ps_acc", tag="ps_acc")
                   for _ in range(2)]
            sp = ps_s.tile([128, HP * QT], F32, name="sp", tag="sp")
            psums = pss + [sp[:, 0:QT], sp[:, QT:2 * QT]]
            for ei in range(EC):
                for j in range(4):
                    eo = eo_base + j
                    nc.tensor.matmul(
                        psums[j],
                        lhsT=wtiles[ei][:, 128 * eo:128 * (eo + 1)],
                        rhs=xT[ei][:, QT * s:QT * (s + 1)],
                        start=(ei == 0),
                        stop=(ei == EC - 1),
                    )
            for j in range(4):
                evict(eo_base + j, psums[j])

        # ---- per-group projection emitters ----
        def proj_group(wtiles, s, eo):
            # one output-feature group of the strip-s projection -> PSUM
            ps = ps_acc.tile([128, QT], F32, name="ps_acc", tag="ps_acc")
            for ei in range(EC):
                nc.tensor.matmul(
                    ps,
                    lhsT=wtiles[ei][:, 128 * eo:128 * (eo + 1)],
                    rhs=xT[ei][:, QT * s:QT * (s + 1)],
                    start=(ei == 0),
                    stop=(ei == EC - 1),
                )
            return ps

        def q_group(s, eo, on_vector=False):
            ps = proj_group(wq, s, eo)
            o = qTp.tile([128, QT], BF16, name="qT", tag="qT")
            if on_vector:
                nc.vector.tensor_copy(out=o, in_=ps)
            else:
                nc.scalar.copy(out=o, in_=ps)
            return o

        def k_group(s, eo, on_vector=False):
            ps = proj_group(wk, s, eo)
            dst = kT[eo][:, QT * s:QT * (s + 1)]
            if on_vector:
                nc.vector.tensor_copy(out=dst, in_=ps)
            else:
                nc.scalar.copy(out=dst, in_=ps)

        kT = [kTp.tile([128, TT], BF16, name="kT", tag="kT") for _ in range(EC)]
        vaug = [vp.tile([128, HH * (Dh + 1)], BF16, name="vaug", tag="vaug")
                for _ in range(TC)]

        def v_group(ti, eoq, on_vector=False):
            va = vaug[ti]
            ps = ps_acc.tile([128, QE], F32, name="ps_acc", tag="ps_acc")
            for ei in range(EC):
                nc.tensor.matmul(
                    ps,
                    lhsT=xT[ei][:, 128 * ti:128 * (ti + 1)],
                    rhs=wv[ei][:, QE * eoq:QE * (eoq + 1)],
                    start=(ei == 0),
                    stop=(ei == EC - 1),
                )
            hq = QE // Dh
            dst = va[:, (Dh + 1) * hq * eoq:(Dh + 1) * hq * (eoq + 1)]
            dst = dst.rearrange("p (h c) -> p h c", c=Dh + 1)[:, :, 0:Dh]
            src = ps.rearrange("p (h d) -> p h d", d=Dh)
            if on_vector:
                nc.vector.tensor_copy(out=dst, in_=src)
            else:
                nc.scalar.copy(out=dst, in_=src)

        aoutT = [aoutp.tile([128, TT], BF16, name="aoutT", tag="aoutT")
                 for _ in range(EC)]

        # ---- attention for one strip; dense_work interleaves PE filler ----
        def attention_strip(s, qTs, dense_work):
            t2max = CS * (s + 1)
            for pi in range(EC):
                opss = [ps_o.tile([Dh + 1, QT], F32, name="ops", tag="ops")
                        for _ in range(HP)]
                pending = []

                def emit_pv(blk):
                    pt, c0, t2 = blk
                    for hi in range(HP):
                        h = HP * pi + hi
                        va_h = vaug[t2][:, h * (Dh + 1):(h + 1) * (Dh + 1)]
                        nc.tensor.matmul(
                            opss[hi][:, c0:QT],
                            lhsT=va_h,
                            rhs=pt[:, QT * hi + c0:QT * (hi + 1)],
                            start=(t2 == 0),
                            stop=(t2 == t2max - 1),
                        )

                for t2 in range(t2max):
                    k0 = 128 * t2 - QT * s
                    c0 = max(0, k0)
                    sp = ps_s.tile([128, HP * QT], F32, name="sp", tag="sp")
                    for hi in range(HP):
                        po = Dh * hi
                        nc.tensor.matmul(
                            sp[:, QT * hi + c0:QT * (hi + 1)],
                            lhsT=kT[pi][po:po + Dh, 128 * t2:128 * (t2 + 1)],
                            rhs=qTs[pi][po:po + Dh, c0:QT],
                            start=True,
                            stop=True,
                        )
                    pt = ptp.tile([128, HP * QT], BF16, name="pt", tag="pt")
                    spv = sp[:, :].rearrange("p (h c) -> p h c", c=QT)[:, :, c0:QT]
                    ptv = pt[:, :].rearrange("p (h c) -> p h c", c=QT)[:, :, c0:QT]
                    nc.scalar.activation(out=ptv, in_=spv, func=Exp, scale=scale)
                    if k0 >= 0:
                        # zero the future half of the diagonal 128-block for
                        # both heads in one strided gpsimd select
                        dv = pt[:, :].rearrange("p (h c) -> p h c", c=QT)
                        dv = dv[:, :, c0:c0 + 128]
                        nc.gpsimd.affine_select(
                            out=dv, in_=dv,
                            compare_op=mybir.AluOpType.is_ge, fill=0.0,
                            base=0, pattern=[[0, HP], [1, 128]],
                            channel_multiplier=-1,
                        )
                    pending.append((pt, c0, t2))
                    if len(pending) > 1:
                        emit_pv(pending.pop(0))
                for blk in pending:
                    emit_pv(blk)

                # ---- fast PSUM eviction: raw out + sum rows; frees ps_o ----
                aslices = []
                sums_t = []
                for hi in range(HP):
                    aslice = aoutT[pi][Dh * hi:Dh * (hi + 1), QT * s:QT * (s + 1)]
                    nc.vector.tensor_copy(out=aslice, in_=opss[hi][0:Dh, :])
                    aslices.append(aslice)
                    sums = sumsp.tile([1, QT], BF16, name="sums", tag="sums")
                    seng = nc.scalar if hi == 0 else nc.vector
                    if hi == 0:
                        seng.copy(out=sums, in_=opss[hi][Dh:Dh + 1, :])
                    else:
                        seng.tensor_copy(out=sums, in_=opss[hi][Dh:Dh + 1, :])
                    sums_t.append(sums)
                # dense filler keeps the PE busy while normalization trails
                if dense_work is not None:
                    dense_work(pi)
                # ---- deferred normalization: rank-1 broadcast on the PE ----
                # rb_raw[64*hi + p, q] = sums_hi[q]; one reciprocal covers both
                rb_raw = ps_o.tile([128, QT], F32, name="rb_raw", tag="ops")
                for hi in range(HP):
                    nc.tensor.matmul(
                        rb_raw[Dh * hi:Dh * (hi + 1), :],
                        lhsT=ones_t[:, 0:Dh],
                        rhs=sums_t[hi],
                        start=True, stop=True,
                    )
                rbs = rbp.tile([128, QT], F32, name="rbs", tag="rbs")
                nc.vector.tensor_copy(out=rbs, in_=rb_raw)
                rb = rbp.tile([128, QT], F32, name="rb", tag="rb")
                nc.vector.reciprocal_approx_fast(out=rb, in_=rbs)
                for hi in range(HP):
                    nc.vector.tensor_mul(
                        out=aslices[hi], in0=aslices[hi],
                        in1=rb[Dh * hi:Dh * (hi + 1), :],
                    )

        # ---- out-projection / FFN emitters (per strip, chunked) ----
        def outproj_eo(wo, s, eo):
            ps = ps_acc.tile([128, QT], F32, name="ps_acc", tag="ps_acc")
            for ei in range(EC):
                nc.tensor.matmul(
                    ps,
                    lhsT=wo[ei][:, 128 * eo:128 * (eo + 1)],
                    rhs=aoutT[ei][:, QT * s:QT * (s + 1)],
                    start=(ei == 0),
                    stop=(ei == EC - 1),
                )
            nc.scalar.activation(
                out=projT[eo][:, QT * s:QT * (s + 1)], in_=ps,
                func=Ident, bias=bo_sb[:, eo:eo + 1], scale=1.0,
            )

        def ffn_group(w1, s, g):
            # g in [0, 2*CS): token chunk = s*CS + g//NE, eoq = g%NE
            ti = CS * s + g // NE
            eoq = g % NE
            ps = ps_acc.tile([128, QE], F32, name="ps_acc", tag="ps_acc")
            for ei in range(EC):
                nc.tensor.matmul(
                    ps,
                    lhsT=projT[ei][:, 128 * ti:128 * (ti + 1)],
                    rhs=w1[ei][:, QE * eoq:QE * (eoq + 1)],
                    start=(ei == 0),
                    stop=False,
                )
            nc.tensor.matmul(
                ps,
                lhsT=ones_t[:, 0:128],
                rhs=b1_sb[:, QE * eoq:QE * (eoq + 1)],
                start=False,
                stop=True,
            )
            fo = ffoutp.tile([128, QE], F32)
            nc.scalar.activation(out=fo, in_=ps, func=Relu)
            # split the store across two DMA queues, rotating per group;
            # the final two groups use the two HWDGE queues so the tail
            # isn't gated by the slower SWDGE drain
            half = QE // 2
            if s == NS - 1 and g >= 2 * CS - 2:
                qs = (nc.sync, nc.scalar)
            else:
                qs = [(nc.sync, nc.gpsimd), (nc.gpsimd, nc.sync)][g % 2]
            for z, qeng in enumerate(qs):
                qeng.dma_start(
                    out=out[128 * ti:128 * (ti + 1),
                            QE * eoq + half * z:QE * eoq + half * (z + 1)],
                    in_=fo[:, half * z:half * (z + 1)],
                )

        # ================= program order =================
        # strip 0 inputs
        xtoks = {}
        for ti in range(TC):
            xtoks[ti] = stage_x(ti)
            if ti < CS:
                transpose_x(ti, xtoks[ti])
        for ti in range(CS):
            nc.gpsimd.memset(vaug[ti], 1.0)

        qT0 = [None] * EC

        def q_evict(eo, ps):
            o = qTp.tile([128, QT], BF16, name="qT", tag="qT")
            nc.scalar.copy(out=o, in_=ps)
            qT0[eo] = o

        def k_evict(eo, ps):
            nc.scalar.copy(out=kT[eo][:, 0:QT], in_=ps)

        proj_half_paced(wq, 0, 0, q_evict)
        proj_half_paced(wq, 0, 4, q_evict)
        proj_half_paced(wk, 0, 0, k_evict)
        proj_half_paced(wk, 0, 4, k_evict)
        for ti in range(CS):
            for eoq in range(NE):
                v_group(ti, eoq)
        wo = load_w(Wo)
        w1 = load_w(W1)
        for ti in range(CS, TC):
            nc.gpsimd.memset(vaug[ti], 1.0)
        for ti in range(CS, TC):
            transpose_x(ti, xtoks[ti])
        projT = [projp.tile([128, TT], BF16, name="projT", tag="projT")
                 for _ in range(EC)]

        # attention strip 0 with strip-1 Q/K/V interleaved between pairs
        qT1 = [None] * EC

        def qkv1_filler(pi):
            qT1[pi] = q_group(1, pi, on_vector=True)
            k_group(1, pi, on_vector=True)
            v_group(CS + pi // NE, pi % NE, on_vector=True)

        attention_strip(0, qT0, qkv1_filler)

        # attention strip 1 with strip-0 out-proj/FFN interleaved
        def dense_filler(pi):
            if pi < 4:
                outproj_eo(wo, 0, 2 * pi)
                outproj_eo(wo, 0, 2 * pi + 1)
            else:
                ffn_group(w1, 0, 2 * (pi - 4))
                ffn_group(w1, 0, 2 * (pi - 4) + 1)

        attention_strip(1, qT1, dense_filler)

        for eo in range(EC):
            outproj_eo(wo, 1, eo)
        for g in range(2 * CS):
            ffn_group(w1, 1, g)

    nc.finalize()
    return nc


_NC_CACHE = {}


def _get_nc(shape_key):
    if shape_key not in _NC_CACHE:
        _NC_CACHE[shape_key] = build_nc(*shape_key)
    return _NC_CACHE[shape_key]


def kernel(x, Wq, Wk, Wv, Wo, bo, W1, b1):
    x = np.ascontiguousarray(np.asarray(x, dtype=np.float32))
    ws = {
        "Wq": np.ascontiguousarray(np.asarray(Wq, dtype=np.float32)),
        "Wk": np.ascontiguousarray(np.asarray(Wk, dtype=np.float32)),
        "Wv": np.ascontiguousarray(np.asarray(Wv, dtype=np.float32)),
        "Wo": np.ascontiguousarray(np.asarray(Wo, dtype=np.float32)),
        "bo": np.ascontiguousarray(np.asarray(bo, dtype=np.float32)),
        "W1": np.ascontiguousarray(np.asarray(W1, dtype=np.float32)),
        "b1": np.ascontiguousarray(np.asarray(b1, dtype=np.float32)),
    }
    B, TT, EE = x.shape
    assert B == N_CORES
    nc = _get_nc((TT, EE, H, DH))
    in_maps = [dict(ws, x=x[b]) for b in range(B)]
    res = run_bass_kernel_spmd(nc, in_maps, core_ids=list(range(N_CORES)))
    return np.stack([res.results[b]["out"] for b in range(B)], axis=0).astype(
        np.float32
    )


# revision 47
# speedup vs baseline: 1.2054x; 1.0008x over previous
"""Dense transformer block (QKV -> causal attention -> out-proj -> FFN+ReLU)
on 8 Trainium2 NeuronCores, data-parallel over the batch dimension.

Contract: kernel(**inputs) takes the FULL inputs
  x [8, 1024, 1024] f32, Wq/Wk/Wv/Wo/W1 [1024, 1024] f32, bo/b1 [1024] f32
and returns the FULL output [8, 1024, 1024] f32.

Each of the 8 cores runs the identical single-core program on one batch
element (batch=8, cores=8 -> no collectives needed).

Single-core design (bf16 tensor-engine compute, fp32 accumulation), v3:
  - The token dim is processed in two strips of 512 queries so that
    early phases overlap late ones: transposes/QKV of strip 0 ->
    attention strip 0 (with the strip-1 Q/K/V projections interleaved
    between its head pairs) -> attention strip 1 (with the strip-0
    output projection and FFN interleaved) -> out-proj/FFN strip 1.
  - The first two GEMMs (Q0/K0) run ei-outer over four output groups
    at once, paced by the weight-tile DMA arrivals, so matmuls start
    before the full weight matrix has landed.
  - Scores use true K=64 matmuls placed in PE quadrants via
    tile_position (even head rows 0-63, odd head rows 64-127); both
    heads of a pair write the two banks of one [128, 1024] PSUM tile
    so a single strided ACT instruction exponentiates the pair.
  - Causal masking of the diagonal 128x128 sub-block: one strided
    gpsimd affine_select per block zeroes the future half of the exp
    output for both heads; fully-future blocks are skipped and
    partially valid blocks only compute their valid column range.
  - V carries an extra all-ones column per head ("augmented V") so the
    softmax denominators fall out of the attn@v matmul as row Dh.
  - Normalization per head pair: raw PSUM eviction (bf16 attention out
    + sum rows) frees PSUM fast; the 64-partition broadcast of the
    sums is a rank-1 ones(x)sums matmul on the PE, inverted by one
    18-bit reciprocal_approx_fast, then multiplied in place on DVE.
  - out-proj emits projT feature-major (= the lhsT the FFN needs) with
    bo fused via the ACT bias port; FFN emits token-major with b1
    folded in via a K=1 ones-row matmul, ReLU on PSUM eviction, and
    stores split across rotating DMA queues.
"""

import numpy as np
from contextlib import ExitStack

import concourse.bass as bass
import concourse.bacc as bacc
import concourse.tile as tile
from concourse import mybir
from concourse.bass_utils import run_bass_kernel_spmd
from concourse.masks import make_identity

F32 = mybir.dt.float32
BF16 = mybir.dt.bfloat16

N_CORES = 8
BATCH = 8
T = 1024
E = 1024
H = 16
DH = 64


def build_nc(TT=T, EE=E, HH=H, Dh=DH):
    nc = bacc.Bacc("TRN2", target_bir_lowering=False, num_swdge_queues=4)

    x = nc.dram_tensor("x", [TT, EE], F32, kind="ExternalInput")
    Wq = nc.dram_tensor("Wq", [EE, EE], F32, kind="ExternalInput")
    Wk = nc.dram_tensor("Wk", [EE, EE], F32, kind="ExternalInput")
    Wv = nc.dram_tensor("Wv", [EE, EE], F32, kind="ExternalInput")
    Wo = nc.dram_tensor("Wo", [EE, EE], F32, kind="ExternalInput")
    bo = nc.dram_tensor("bo", [EE], F32, kind="ExternalInput")
    W1 = nc.dram_tensor("W1", [EE, EE], F32, kind="ExternalInput")
    b1 = nc.dram_tensor("b1", [EE], F32, kind="ExternalInput")
    out = nc.dram_tensor("out", [TT, EE], F32, kind="ExternalOutput")

    EC = EE // 128          # feature-chunk count (partition tiles)
    TC = TT // 128          # token-chunk count
    QT = 512                # query-strip width
    NS = TT // QT           # number of strips (2)
    CS = TC // NS           # token chunks per strip (4)
    QE = 512                # output-feature free-dim chunk for V/FFN
    NE = EE // QE
    HP = 128 // Dh          # heads per 128-partition feature tile (2)
    scale = float(Dh) ** -0.5
    Exp = mybir.ActivationFunctionType.Exp
    Relu = mybir.ActivationFunctionType.Relu
    Ident = mybir.ActivationFunctionType.Identity
    Mult = mybir.AluOpType.mult

    with ExitStack() as ctx:
        tc = ctx.enter_context(tile.TileContext(nc))
        wpool = ctx.enter_context(tc.tile_pool(name="w", bufs=5 * EC))
        xstagep = ctx.enter_context(tc.tile_pool(name="xstage", bufs=2))
        xtokp = ctx.enter_context(tc.tile_pool(name="xtok", bufs=2))
        xTp = ctx.enter_context(tc.tile_pool(name="xT", bufs=EC))
        qTp = ctx.enter_context(tc.tile_pool(name="qT", bufs=EC))
        kTp = ctx.enter_context(tc.tile_pool(name="kT", bufs=EC))
        vp = ctx.enter_context(tc.tile_pool(name="v", bufs=TC))
        ptp = ctx.enter_context(tc.tile_pool(name="pt", bufs=4))
        sumsp = ctx.enter_context(tc.tile_pool(name="sums", bufs=2))
        rbp = ctx.enter_context(tc.tile_pool(name="rb", bufs=2))
        aoutp = ctx.enter_context(tc.tile_pool(name="aout", bufs=EC))
        projp = ctx.enter_context(tc.tile_pool(name="proj", bufs=EC))
        constp = ctx.enter_context(tc.tile_pool(name="const", bufs=1))
        ffoutp = ctx.enter_context(tc.tile_pool(name="ffout", bufs=2))
        ps_acc = ctx.enter_context(tc.tile_pool(name="ps_acc", bufs=2, space="PSUM"))
        ps_s = ctx.enter_context(tc.tile_pool(name="ps_s", bufs=2, space="PSUM"))
        ps_o = ctx.enter_context(tc.tile_pool(name="ps_o", bufs=2, space="PSUM"))

        def load_w(wdram):
            tiles = []
            for ei in range(EC):
                wt = wpool.tile([128, EE], BF16, tag="w")
                nc.gpsimd.dma_start(out=wt, in_=wdram[128 * ei:128 * (ei + 1), :])
                tiles.append(wt)
            return tiles

        # wq first on the gpsimd queue: its arrival gates the first GEMM
        wq = load_w(Wq)

        # ---- constants ----
        bo_sb = constp.tile([128, EC], F32)
        nc.sync.dma_start(out=bo_sb, in_=bo.rearrange("(c p) -> p c", p=128))
        ones_t = constp.tile([1, 128], BF16)
        nc.vector.memset(ones_t, 1.0)
        ident = constp.tile([128, 128], BF16)
        make_identity(nc, ident)
        b1_sb = constp.tile([1, EE], BF16)
        nc.gpsimd.dma_start(out=b1_sb, in_=b1.rearrange("(a e) -> a e", a=1))

        wk = load_w(Wk)
        wv = load_w(Wv)

        # ---- x chunk staging: fp32 DMA -> vector bf16 cast -> PE transpose ----
        xT = [xTp.tile([128, TT], BF16, name="xT", tag="xT") for _ in range(EC)]

        def stage_x(ti):
            xstage = xstagep.tile([128, EE], F32)
            # split by rows (not columns): each half is a fully contiguous
            # DRAM read, so the DMA moves 4KB bursts instead of strided 2KB
            for z, dma_eng in enumerate((nc.sync, nc.scalar)):
                dma_eng.dma_start(
                    out=xstage[64 * z:64 * (z + 1), :],
                    in_=x[128 * ti + 64 * z:128 * ti + 64 * (z + 1), :],
                )
            xtok = xtokp.tile([128, EE], BF16)
            nc.vector.tensor_copy(out=xtok, in_=xstage)
            return xtok

        def transpose_x(ti, xtok):
            for ec in range(EC):
                if ec % 2 == 0:
                    ps_t = ps_acc.tile([128, 128], BF16, name="ps_t", tag="ps_acc")
                else:
                    ps_t = ps_o.tile([128, 128], BF16, name="ps_t", tag="ops")
                nc.tensor.transpose(
                    ps_t, xtok[:, 128 * ec:128 * (ec + 1)], ident
                )
                dst = xT[ec][:, 128 * ti:128 * (ti + 1)]
                if ec % 2 == 0:
                    nc.vector.tensor_copy(out=dst, in_=ps_t)
                else:
                    nc.scalar.copy(out=dst, in_=ps_t)

        # ---- weight-arrival-paced projection (first GEMMs only) ----
        # ei-outer over four output groups at once: matmuls start as soon
        # as the first weight tiles land instead of waiting for all eight.
        def proj_half_paced(wtiles, s, eo_base, evict):
            pss = [ps_acc.tile([128, QT], F32, name="ps_acc", tag="ps_acc")
                   for _ in range(2)]
            sp = ps_s.tile([128, HP * QT], F32, name="sp", tag="sp")
            psums = pss + [sp[:, 0:QT], sp[:, QT:2 * QT]]
            for ei in range(EC):
                for j in range(4):
                    eo = eo_base + j
                    nc.tensor.matmul(
                        psums[j],
                        lhsT=wtiles[ei][:, 128 * eo:128 * (eo + 1)],
                        rhs=xT[ei][:, QT * s:QT * (s + 1)],
                        start=(ei == 0),
                        stop=(ei == EC - 1),
                    )
            for j in range(4):
                evict(eo_base + j, psums[j])

        # ---- per-group projection emitters ----
        def proj_group(wtiles, s, eo):
            # one output-feature group of the strip-s projection -> PSUM
            ps = ps_acc.tile([128, QT], F32, name="ps_acc", tag="ps_acc")
            for ei in range(EC):
                nc.tensor.matmul(
                    ps,
                    lhsT=wtiles[ei][:, 128 * eo:128 * (eo + 1)],
                    rhs=xT[ei][:, QT * s:QT * (s + 1)],
                    start=(ei == 0),
                    stop=(ei == EC - 1),
                )
            return ps

        def q_group(s, eo, on_vector=False):
            ps = proj_group(wq, s, eo)
            o = qTp.tile([128, QT], BF16, name="qT", tag="qT")
            if on_vector:
                nc.vector.tensor_copy(out=o, in_=ps)
            else:
                nc.scalar.copy(out=o, in_=ps)
            return o

        def k_group(s, eo, on_vector=False):
            ps = proj_group(wk, s, eo)
            dst = kT[eo][:, QT * s:QT * (s + 1)]
            if on_vector:
                nc.vector.tensor_copy(out=dst, in_=ps)
            else:
                nc.scalar.copy(out=dst, in_=ps)

        kT = [kTp.tile([128, TT], BF16, name="kT", tag="kT") for _ in range(EC)]
        vaug = [vp.tile([128, HH * (Dh + 1)], BF16, name="vaug", tag="vaug")
                for _ in range(TC)]

        def v_group(ti, eoq, on_vector=False):
            va = vaug[ti]
            ps = ps_acc.tile([128, QE], F32, name="ps_acc", tag="ps_acc")
            for ei in range(EC):
                nc.tensor.matmul(
                    ps,
                    lhsT=xT[ei][:, 128 * ti:128 * (ti + 1)],
                    rhs=wv[ei][:, QE * eoq:QE * (eoq + 1)],
                    start=(ei == 0),
                    stop=(ei == EC - 1),
                )
            hq = QE // Dh
            dst = va[:, (Dh + 1) * hq * eoq:(Dh + 1) * hq * (eoq + 1)]
            dst = dst.rearrange("p (h c) -> p h c", c=Dh + 1)[:, :, 0:Dh]
            src = ps.rearrange("p (h d) -> p h d", d=Dh)
            if on_vector:
                nc.vector.tensor_copy(out=dst, in_=src)
            else:
                nc.scalar.copy(out=dst, in_=src)

        aoutT = [aoutp.tile([128, TT], BF16, name="aoutT", tag="aoutT")
                 for _ in range(EC)]

        # ---- attention for one strip; dense_work interleaves PE filler ----
        def attention_strip(s, qTs, dense_work):
            t2max = CS * (s + 1)
            for pi in range(EC):
                opss = [ps_o.tile([Dh + 1, QT], F32, name="ops", tag="ops")
                        for _ in range(HP)]
                pending = []

                def emit_pv(blk):
                    pt, c0, t2 = blk
                    for hi in range(HP):
                        h = HP * pi + hi
                        va_h = vaug[t2][:, h * (Dh + 1):(h + 1) * (Dh + 1)]
                        nc.tensor.matmul(
                            opss[hi][:, c0:QT],
                            lhsT=va_h,
                            rhs=pt[:, QT * hi + c0:QT * (hi + 1)],
                            start=(t2 == 0),
                            stop=(t2 == t2max - 1),
                        )

                for t2 in range(t2max):
                    k0 = 128 * t2 - QT * s
                    c0 = max(0, k0)
                    sp = ps_s.tile([128, HP * QT], F32, name="sp", tag="sp")
                    for hi in range(HP):
                        po = Dh * hi
                        nc.tensor.matmul(
                            sp[:, QT * hi + c0:QT * (hi + 1)],
                            lhsT=kT[pi][po:po + Dh, 128 * t2:128 * (t2 + 1)],
                            rhs=qTs[pi][po:po + Dh, c0:QT],
                            start=True,
                            stop=True,
                        )
                    pt = ptp.tile([128, HP * QT], BF16, name="pt", tag="pt")
                    spv = sp[:, :].rearrange("p (h c) -> p h c", c=QT)[:, :, c0:QT]
                    ptv = pt[:, :].rearrange("p (h c) -> p h c", c=QT)[:, :, c0:QT]
                    nc.scalar.activation(out=ptv, in_=spv, func=Exp, scale=scale)
                    if k0 >= 0:
                        # zero the future half of the diagonal 128-block for
                        # both heads in one strided gpsimd select
                        dv = pt[:, :].rearrange("p (h c) -> p h c", c=QT)
                        dv = dv[:, :, c0:c0 + 128]
                        nc.gpsimd.affine_select(
                            out=dv, in_=dv,
                            compare_op=mybir.AluOpType.is_ge, fill=0.0,
                            base=0, pattern=[[0, HP], [1, 128]],
                            channel_multiplier=-1,
                        )
                    pending.append((pt, c0, t2))
                    if len(pending) > 1:
                        emit_pv(pending.pop(0))
                for blk in pending:
                    emit_pv(blk)

                # ---- fast PSUM eviction: raw out + sum rows; frees ps_o ----
                aslices = []
                sums_t = []
                for hi in range(HP):
                    aslice = aoutT[pi][Dh * hi:Dh * (hi + 1), QT * s:QT * (s + 1)]
                    nc.vector.tensor_copy(out=aslice, in_=opss[hi][0:Dh, :])
                    aslices.append(aslice)
                    sums = sumsp.tile([1, QT], BF16, name="sums", tag="sums")
                    seng = nc.scalar if hi == 0 else nc.vector
                    if hi == 0:
                        seng.copy(out=sums, in_=opss[hi][Dh:Dh + 1, :])
                    else:
                        seng.tensor_copy(out=sums, in_=opss[hi][Dh:Dh + 1, :])
                    sums_t.append(sums)
                # dense filler keeps the PE busy while normalization trails
                if dense_work is not None:
                    dense_work(pi)
                # ---- deferred normalization: rank-1 broadcast on the PE ----
                # rb_raw[64*hi + p, q] = sums_hi[q]; one reciprocal covers both
                rb_raw = ps_o.tile([128, QT], F32, name="rb_raw", tag="ops")
                for hi in range(HP):
                    nc.tensor.matmul(
                        rb_raw[Dh * hi:Dh * (hi + 1), :],
                        lhsT=ones_t[:, 0:Dh],
                        rhs=sums_t[hi],
                        start=True, stop=True,
                    )
                rbs = rbp.tile([128, QT], F32, name="rbs", tag="rbs")
                nc.vector.tensor_copy(out=rbs, in_=rb_raw)
                rb = rbp.tile([128, QT], F32, name="rb", tag="rb")
                nc.vector.reciprocal_approx_fast(out=rb, in_=rbs)
                for hi in range(HP):
                    nc.vector.tensor_mul(
                        out=aslices[hi], in0=aslices[hi],
                        in1=rb[Dh * hi:Dh * (hi + 1), :],
                    )

        # ---- out-projection / FFN emitters (per strip, chunked) ----
        def outproj_eo(wo, s, eo):
            ps = ps_acc.tile([128, QT], F32, name="ps_acc", tag="ps_acc")
            for ei in range(EC):
                nc.tensor.matmul(
                    ps,
                    lhsT=wo[ei][:, 128 * eo:128 * (eo + 1)],
                    rhs=aoutT[ei][:, QT * s:QT * (s + 1)],
                    start=(ei == 0),
                    stop=(ei == EC - 1),
                )
            nc.scalar.activation(
                out=projT[eo][:, QT * s:QT * (s + 1)], in_=ps,
                func=Ident, bias=bo_sb[:, eo:eo + 1], scale=1.0,
            )

        def ffn_group(w1, s, g):
            # g in [0, 2*CS): token chunk = s*CS + g//NE, eoq = g%NE
            ti = CS * s + g // NE
            eoq = g % NE
            ps = ps_acc.tile([128, QE], F32, name="ps_acc", tag="ps_acc")
            for ei in range(EC):
                nc.tensor.matmul(
                    ps,
                    lhsT=projT[ei][:, 128 * ti:128 * (ti + 1)],
                    rhs=w1[ei][:, QE * eoq:QE * (eoq + 1)],
                    start=(ei == 0),
                    stop=False,
                )
            nc.tensor.matmul(
                ps,
                lhsT=ones_t[:, 0:128],
                rhs=b1_sb[:, QE * eoq:QE * (eoq + 1)],
                start=False,
                stop=True,
            )
            fo = ffoutp.tile([128, QE], F32)
            nc.scalar.activation(out=fo, in_=ps, func=Relu)
            # split the store across two DMA queues, rotating per group;
            # the final two groups use the two HWDGE queues so the tail
            # isn't gated by the slower SWDGE drain
            half = QE // 2
            if s == NS - 1 and g >= 2 * CS - 2:
                qs = (nc.sync, nc.scalar)
            else:
                qs = [(nc.sync, nc.gpsimd), (nc.gpsimd, nc.sync)][g % 2]
            for z, qeng in enumerate(qs):
                qeng.dma_start(
                    out=out[128 * ti:128 * (ti + 1),
                            QE * eoq + half * z:QE * eoq + half * (z + 1)],
                    in_=fo[:, half * z:half * (z + 1)],
                )

        # ================= program order =================
        # strip 0 inputs
        xtoks = {}
        for ti in range(TC):
            xtoks[ti] = stage_x(ti)
            if ti < CS:
                transpose_x(ti, xtoks[ti])
        for ti in range(CS):
            nc.gpsimd.memset(vaug[ti], 1.0)

        qT0 = [None] * EC

        def q_evict(eo, ps):
            o = qTp.tile([128, QT], BF16, name="qT", tag="qT")
            nc.scalar.copy(out=o, in_=ps)
            qT0[eo] = o

        def k_evict(eo, ps):
            nc.scalar.copy(out=kT[eo][:, 0:QT], in_=ps)

        proj_half_paced(wq, 0, 0, q_evict)
        proj_half_paced(wq, 0, 4, q_evict)
        proj_half_paced(wk, 0, 0, k_evict)
        proj_half_paced(wk, 0, 4, k_evict)
        for ti in range(CS):
            for eoq in range(NE):
                v_group(ti, eoq)
        wo = load_w(Wo)
        w1 = load_w(W1)
        for ti in range(CS, TC):
            nc.gpsimd.memset(vaug[ti], 1.0)
        for ti in range(CS, TC):
            transpose_x(ti, xtoks[ti])
        projT = [projp.tile([128, TT], BF16, name="projT", tag="projT")
                 for _ in range(EC)]

        # attention strip 0 with strip-1 Q/K/V interleaved between pairs
        qT1 = [None] * EC

        def qkv1_filler(pi):
            qT1[pi] = q_group(1, pi, on_vector=True)
            k_group(1, pi, on_vector=True)
            v_group(CS + pi // NE, pi % NE, on_vector=True)

        attention_strip(0, qT0, qkv1_filler)

        # attention strip 1 with strip-0 out-proj/FFN interleaved
        def dense_filler(pi):
            if pi < 4:
                outproj_eo(wo, 0, 2 * pi)
                outproj_eo(wo, 0, 2 * pi + 1)
            else:
                ffn_group(w1, 0, 2 * (pi - 4))
                ffn_group(w1, 0, 2 * (pi - 4) + 1)

        attention_strip(1, qT1, dense_filler)

        for eo in range(EC):
            outproj_eo(wo, 1, eo)
        for g in range(2 * CS):
            ffn_group(w1, 1, g)

    nc.finalize()
    return nc


_NC_CACHE = {}


def _get_nc(shape_key):
    if shape_key not in _NC_CACHE:
        _NC_CACHE[shape_key] = build_nc(*shape_key)
    return _NC_CACHE[shape_key]


def kernel(x, Wq, Wk, Wv, Wo, bo, W1, b1):
    x = np.ascontiguousarray(np.asarray(x, dtype=np.float32))
    ws = {
        "Wq": np.ascontiguousarray(np.asarray(Wq, dtype=np.float32)),
        "Wk": np.ascontiguousarray(np.asarray(Wk, dtype=np.float32)),
        "Wv": np.ascontiguousarray(np.asarray(Wv, dtype=np.float32)),
        "Wo": np.ascontiguousarray(np.asarray(Wo, dtype=np.float32)),
        "bo": np.ascontiguousarray(np.asarray(bo, dtype=np.float32)),
        "W1": np.ascontiguousarray(np.asarray(W1, dtype=np.float32)),
        "b1": np.ascontiguousarray(np.asarray(b1, dtype=np.float32)),
    }
    B, TT, EE = x.shape
    assert B == N_CORES
    nc = _get_nc((TT, EE, H, DH))
    in_maps = [dict(ws, x=x[b]) for b in range(B)]
    res = run_bass_kernel_spmd(nc, in_maps, core_ids=list(range(N_CORES)))
    return np.stack([res.results[b]["out"] for b in range(B)], axis=0).astype(
        np.float32
    )


# revision 49
# speedup vs baseline: 1.2108x; 1.0044x over previous
"""Dense transformer block (QKV -> causal attention -> out-proj -> FFN+ReLU)
on 8 Trainium2 NeuronCores, data-parallel over the batch dimension.

Contract: kernel(**inputs) takes the FULL inputs
  x [8, 1024, 1024] f32, Wq/Wk/Wv/Wo/W1 [1024, 1024] f32, bo/b1 [1024] f32
and returns the FULL output [8, 1024, 1024] f32.

Each of the 8 cores runs the identical single-core program on one batch
element (batch=8, cores=8 -> no collectives needed).

Single-core design (bf16 tensor-engine compute, fp32 accumulation), v3:
  - The token dim is processed in two strips of 512 queries so that
    early phases overlap late ones: transposes/QKV of strip 0 ->
    attention strip 0 (with the strip-1 Q/K/V projections interleaved
    between its head pairs) -> attention strip 1 (with the strip-0
    output projection and FFN interleaved) -> out-proj/FFN strip 1.
  - The first two GEMMs (Q0/K0) run ei-outer over four output groups
    at once, paced by the weight-tile DMA arrivals, so matmuls start
    before the full weight matrix has landed.
  - Scores use true K=64 matmuls placed in PE quadrants via
    tile_position (even head rows 0-63, odd head rows 64-127); both
    heads of a pair write the two banks of one [128, 1024] PSUM tile
    so a single strided ACT instruction exponentiates the pair.
  - Causal masking of the diagonal 128x128 sub-block: one strided
    gpsimd affine_select per block zeroes the future half of the exp
    output for both heads; fully-future blocks are skipped and
    partially valid blocks only compute their valid column range.
  - V carries an extra all-ones column per head ("augmented V") so the
    softmax denominators fall out of the attn@v matmul as row Dh.
  - Normalization per head pair: raw PSUM eviction (bf16 attention out
    + sum rows) frees PSUM fast; the 64-partition broadcast of the
    sums is a rank-1 ones(x)sums matmul on the PE, inverted by one
    18-bit reciprocal_approx_fast, then multiplied in place on DVE.
  - out-proj emits projT feature-major (= the lhsT the FFN needs) with
    bo fused via the ACT bias port; FFN emits token-major with b1
    folded in via a K=1 ones-row matmul, ReLU on PSUM eviction, and
    stores split across rotating DMA queues.
"""

import numpy as np
from contextlib import ExitStack

import concourse.bass as bass
import concourse.bacc as bacc
import concourse.tile as tile
from concourse import mybir
from concourse.bass_utils import run_bass_kernel_spmd
from concourse.masks import make_identity

F32 = mybir.dt.float32
BF16 = mybir.dt.bfloat16

N_CORES = 8
BATCH = 8
T = 1024
E = 1024
H = 16
DH = 64


def build_nc(TT=T, EE=E, HH=H, Dh=DH):
    nc = bacc.Bacc("TRN2", target_bir_lowering=False, num_swdge_queues=4)

    x = nc.dram_tensor("x", [TT, EE], F32, kind="ExternalInput")
    Wq = nc.dram_tensor("Wq", [EE, EE], F32, kind="ExternalInput")
    Wk = nc.dram_tensor("Wk", [EE, EE], F32, kind="ExternalInput")
    Wv = nc.dram_tensor("Wv", [EE, EE], F32, kind="ExternalInput")
    Wo = nc.dram_tensor("Wo", [EE, EE], F32, kind="ExternalInput")
    bo = nc.dram_tensor("bo", [EE], F32, kind="ExternalInput")
    W1 = nc.dram_tensor("W1", [EE, EE], F32, kind="ExternalInput")
    b1 = nc.dram_tensor("b1", [EE], F32, kind="ExternalInput")
    out = nc.dram_tensor("out", [TT, EE], F32, kind="ExternalOutput")

    EC = EE // 128          # feature-chunk count (partition tiles)
    TC = TT // 128          # token-chunk count
    QT = 512                # query-strip width
    NS = TT // QT           # number of strips (2)
    CS = TC // NS           # token chunks per strip (4)
    QE = 512                # output-feature free-dim chunk for V/FFN
    NE = EE // QE
    HP = 128 // Dh          # heads per 128-partition feature tile (2)
    scale = float(Dh) ** -0.5
    Exp = mybir.ActivationFunctionType.Exp
    Relu = mybir.ActivationFunctionType.Relu
    Ident = mybir.ActivationFunctionType.Identity
    Mult = mybir.AluOpType.mult

    with ExitStack() as ctx:
        tc = ctx.enter_context(tile.TileContext(nc))
        wpool = ctx.enter_context(tc.tile_pool(name="w", bufs=5 * EC))
        xstagep = ctx.enter_context(tc.tile_pool(name="xstage", bufs=2))
        xtokp = ctx.enter_context(tc.tile_pool(name="xtok", bufs=2))
        xTp = ctx.enter_context(tc.tile_pool(name="xT", bufs=EC))
        qTp = ctx.enter_context(tc.tile_pool(name="qT", bufs=EC))
        kTp = ctx.enter_context(tc.tile_pool(name="kT", bufs=EC))
        vp = ctx.enter_context(tc.tile_pool(name="v", bufs=TC))
        ptp = ctx.enter_context(tc.tile_pool(name="pt", bufs=4))
        sumsp = ctx.enter_context(tc.tile_pool(name="sums", bufs=2))
        rbp = ctx.enter_context(tc.tile_pool(name="rb", bufs=2))
        aoutp = ctx.enter_context(tc.tile_pool(name="aout", bufs=EC))
        projp = ctx.enter_context(tc.tile_pool(name="proj", bufs=EC))
        constp = ctx.enter_context(tc.tile_pool(name="const", bufs=1))
        ffoutp = ctx.enter_context(tc.tile_pool(name="ffout", bufs=2))
        ps_acc = ctx.enter_context(tc.tile_pool(name="ps_acc", bufs=2, space="PSUM"))
        ps_s = ctx.enter_context(tc.tile_pool(name="ps_s", bufs=2, space="PSUM"))
        ps_o = ctx.enter_context(tc.tile_pool(name="ps_o", bufs=2, space="PSUM"))

        def load_w(wdram):
            tiles = []
            for ei in range(EC):
                wt = wpool.tile([128, EE], BF16, tag="w")
                nc.gpsimd.dma_start(out=wt, in_=wdram[128 * ei:128 * (ei + 1), :])
                tiles.append(wt)
            return tiles

        # wq first on the gpsimd queue: its arrival gates the first GEMM
        wq = load_w(Wq)

        # ---- constants ----
        bo_sb = constp.tile([128, EC], F32)
        nc.sync.dma_start(out=bo_sb, in_=bo.rearrange("(c p) -> p c", p=128))
        ones_t = constp.tile([1, 128], BF16)
        nc.vector.memset(ones_t, 1.0)
        ident = constp.tile([128, 128], BF16)
        make_identity(nc, ident)
        b1_sb = constp.tile([1, EE], BF16)
        nc.gpsimd.dma_start(out=b1_sb, in_=b1.rearrange("(a e) -> a e", a=1))

        wk = load_w(Wk)
        wv = load_w(Wv)

        # ---- x chunk staging: fp32 DMA -> vector bf16 cast -> PE transpose ----
        xT = [xTp.tile([128, TT], BF16, name="xT", tag="xT") for _ in range(EC)]

        def stage_x(ti):
            xstage = xstagep.tile([128, EE], F32)
            # split by rows (not columns): each half is a fully contiguous
            # DRAM read, so the DMA moves 4KB bursts instead of strided 2KB
            for z, dma_eng in enumerate((nc.sync, nc.scalar)):
                dma_eng.dma_start(
                    out=xstage[64 * z:64 * (z + 1), :],
                    in_=x[128 * ti + 64 * z:128 * ti + 64 * (z + 1), :],
                )
            xtok = xtokp.tile([128, EE], BF16)
            nc.vector.tensor_copy(out=xtok, in_=xstage)
            return xtok

        def transpose_x(ti, xtok):
            for ec in range(EC):
                if ec % 2 == 0:
                    ps_t = ps_acc.tile([128, 128], BF16, name="ps_t", tag="ps_acc")
                else:
                    ps_t = ps_o.tile([128, 128], BF16, name="ps_t", tag="ops")
                nc.tensor.transpose(
                    ps_t, xtok[:, 128 * ec:128 * (ec + 1)], ident
                )
                dst = xT[ec][:, 128 * ti:128 * (ti + 1)]
                if ec % 2 == 0:
                    nc.vector.tensor_copy(out=dst, in_=ps_t)
                else:
                    nc.scalar.copy(out=dst, in_=ps_t)

        # ---- weight-arrival-paced projection (first GEMMs only) ----
        # ei-outer over four output groups at once: matmuls start as soon
        # as the first weight tiles land instead of waiting for all eight.
        def proj_half_paced(wtiles, s, eo_base, evict):
            pss = [ps_acc.tile([128, QT], F32, name="ps_acc", tag="ps_acc")
                   for _ in range(2)]
            sp = ps_s.tile([128, HP * QT], F32, name="sp", tag="sp")
            psums = pss + [sp[:, 0:QT], sp[:, QT:2 * QT]]
            for ei in range(EC):
                for j in range(4):
                    eo = eo_base + j
                    nc.tensor.matmul(
                        psums[j],
                        lhsT=wtiles[ei][:, 128 * eo:128 * (eo + 1)],
                        rhs=xT[ei][:, QT * s:QT * (s + 1)],
                        start=(ei == 0),
                        stop=(ei == EC - 1),
                    )
            for j in range(4):
                evict(eo_base + j, psums[j])

        # ---- per-group projection emitters ----
        def proj_group(wtiles, s, eo):
            # one output-feature group of the strip-s projection -> PSUM
            ps = ps_acc.tile([128, QT], F32, name="ps_acc", tag="ps_acc")
            for ei in range(EC):
                nc.tensor.matmul(
                    ps,
                    lhsT=wtiles[ei][:, 128 * eo:128 * (eo + 1)],
                    rhs=xT[ei][:, QT * s:QT * (s + 1)],
                    start=(ei == 0),
                    stop=(ei == EC - 1),
                )
            return ps

        def q_group(s, eo, on_vector=False):
            ps = proj_group(wq, s, eo)
            o = qTp.tile([128, QT], BF16, name="qT", tag="qT")
            if on_vector:
                nc.vector.tensor_copy(out=o, in_=ps)
            else:
                nc.scalar.copy(out=o, in_=ps)
            return o

        def k_group(s, eo, on_vector=False):
            ps = proj_group(wk, s, eo)
            dst = kT[eo][:, QT * s:QT * (s + 1)]
            if on_vector:
                nc.vector.tensor_copy(out=dst, in_=ps)
            else:
                nc.scalar.copy(out=dst, in_=ps)

        kT = [kTp.tile([128, TT], BF16, name="kT", tag="kT") for _ in range(EC)]
        vaug = [vp.tile([128, HH * (Dh + 1)], BF16, name="vaug", tag="vaug")
                for _ in range(TC)]

        def v_group(ti, eoq, on_vector=False):
            va = vaug[ti]
            ps = ps_acc.tile([128, QE], F32, name="ps_acc", tag="ps_acc")
            for ei in range(EC):
                nc.tensor.matmul(
                    ps,
                    lhsT=xT[ei][:, 128 * ti:128 * (ti + 1)],
                    rhs=wv[ei][:, QE * eoq:QE * (eoq + 1)],
                    start=(ei == 0),
                    stop=(ei == EC - 1),
                )
            hq = QE // Dh
            dst = va[:, (Dh + 1) * hq * eoq:(Dh + 1) * hq * (eoq + 1)]
            dst = dst.rearrange("p (h c) -> p h c", c=Dh + 1)[:, :, 0:Dh]
            src = ps.rearrange("p (h d) -> p h d", d=Dh)
            if on_vector:
                nc.vector.tensor_copy(out=dst, in_=src)
            else:
                nc.scalar.copy(out=dst, in_=src)

        aoutT = [aoutp.tile([128, TT], BF16, name="aoutT", tag="aoutT")
                 for _ in range(EC)]

        # ---- attention for one strip; dense_work interleaves PE filler ----
        def attention_strip(s, qTs, dense_work):
            t2max = CS * (s + 1)
            for pi in range(EC):
                opss = [ps_o.tile([Dh + 1, QT], F32, name="ops", tag="ops")
                        for _ in range(HP)]
                pending = []

                def emit_pv(blk):
                    pt, c0, t2 = blk
                    for hi in range(HP):
                        h = HP * pi + hi
                        va_h = vaug[t2][:, h * (Dh + 1):(h + 1) * (Dh + 1)]
                        nc.tensor.matmul(
                            opss[hi][:, c0:QT],
                            lhsT=va_h,
                            rhs=pt[:, QT * hi + c0:QT * (hi + 1)],
                            start=(t2 == 0),
                            stop=(t2 == t2max - 1),
                        )

                for t2 in range(t2max):
                    k0 = 128 * t2 - QT * s
                    c0 = max(0, k0)
                    sp = ps_s.tile([128, HP * QT], F32, name="sp", tag="sp")
                    for hi in range(HP):
                        po = Dh * hi
                        nc.tensor.matmul(
                            sp[:, QT * hi + c0:QT * (hi + 1)],
                            lhsT=kT[pi][po:po + Dh, 128 * t2:128 * (t2 + 1)],
                            rhs=qTs[pi][po:po + Dh, c0:QT],
                            start=True,
                            stop=True,
                        )
                    pt = ptp.tile([128, HP * QT], BF16, name="pt", tag="pt")
                    spv = sp[:, :].rearrange("p (h c) -> p h c", c=QT)[:, :, c0:QT]
                    ptv = pt[:, :].rearrange("p (h c) -> p h c", c=QT)[:, :, c0:QT]
                    nc.scalar.activation(out=ptv, in_=spv, func=Exp, scale=scale)
                    if k0 >= 0:
                        # zero the future half of the diagonal 128-block for
                        # both heads in one strided gpsimd select
                        dv = pt[:, :].rearrange("p (h c) -> p h c", c=QT)
                        dv = dv[:, :, c0:c0 + 128]
                        nc.gpsimd.affine_select(
                            out=dv, in_=dv,
                            compare_op=mybir.AluOpType.is_ge, fill=0.0,
                            base=0, pattern=[[0, HP], [1, 128]],
                            channel_multiplier=-1,
                        )
                    pending.append((pt, c0, t2))
                    if len(pending) > 1:
                        emit_pv(pending.pop(0))
                for blk in pending:
                    emit_pv(blk)

                # ---- fast PSUM eviction: raw out + sum rows; frees ps_o ----
                aslices = []
                sums_t = []
                for hi in range(HP):
                    aslice = aoutT[pi][Dh * hi:Dh * (hi + 1), QT * s:QT * (s + 1)]
                    nc.vector.tensor_copy(out=aslice, in_=opss[hi][0:Dh, :])
                    aslices.append(aslice)
                    sums = sumsp.tile([1, QT], BF16, name="sums", tag="sums")
                    seng = nc.scalar if hi == 0 else nc.vector
                    if hi == 0:
                        seng.copy(out=sums, in_=opss[hi][Dh:Dh + 1, :])
                    else:
                        seng.tensor_copy(out=sums, in_=opss[hi][Dh:Dh + 1, :])
                    sums_t.append(sums)
                # dense filler keeps the PE busy while normalization trails
                if dense_work is not None:
                    dense_work(pi)
                # ---- deferred normalization: rank-1 broadcast on the PE ----
                # rb_raw[64*hi + p, q] = sums_hi[q]; one reciprocal covers both
                rb_raw = ps_o.tile([128, QT], F32, name="rb_raw", tag="ops")
                for hi in range(HP):
                    nc.tensor.matmul(
                        rb_raw[Dh * hi:Dh * (hi + 1), :],
                        lhsT=ones_t[:, 0:Dh],
                        rhs=sums_t[hi],
                        start=True, stop=True,
                    )
                rbs = rbp.tile([128, QT], F32, name="rbs", tag="rbs")
                nc.vector.tensor_copy(out=rbs, in_=rb_raw)
                rb = rbp.tile([128, QT], F32, name="rb", tag="rb")
                nc.vector.reciprocal_approx_fast(out=rb, in_=rbs)
                for hi in range(HP):
                    nc.vector.tensor_mul(
                        out=aslices[hi], in0=aslices[hi],
                        in1=rb[Dh * hi:Dh * (hi + 1), :],
                    )

        # ---- out-projection / FFN emitters (per strip, chunked) ----
        def outproj_eo(wo, s, eo):
            ps = ps_acc.tile([128, QT], F32, name="ps_acc", tag="ps_acc")
            for ei in range(EC):
                nc.tensor.matmul(
                    ps,
                    lhsT=wo[ei][:, 128 * eo:128 * (eo + 1)],
                    rhs=aoutT[ei][:, QT * s:QT * (s + 1)],
                    start=(ei == 0),
                    stop=(ei == EC - 1),
                )
            nc.scalar.activation(
                out=projT[eo][:, QT * s:QT * (s + 1)], in_=ps,
                func=Ident, bias=bo_sb[:, eo:eo + 1], scale=1.0,
            )

        def ffn_group(w1, s, g):
            # g in [0, 2*CS): token chunk = s*CS + g//NE, eoq = g%NE
            ti = CS * s + g // NE
            eoq = g % NE
            ps = ps_acc.tile([128, QE], F32, name="ps_acc", tag="ps_acc")
            for ei in range(EC):
                nc.tensor.matmul(
                    ps,
                    lhsT=projT[ei][:, 128 * ti:128 * (ti + 1)],
                    rhs=w1[ei][:, QE * eoq:QE * (eoq + 1)],
                    start=(ei == 0),
                    stop=False,
                )
            nc.tensor.matmul(
                ps,
                lhsT=ones_t[:, 0:128],
                rhs=b1_sb[:, QE * eoq:QE * (eoq + 1)],
                start=False,
                stop=True,
            )
            fo = ffoutp.tile([128, QE], F32)
            nc.scalar.activation(out=fo, in_=ps, func=Relu)
            # split the store across two DMA queues, rotating per group;
            # the final two groups use the two HWDGE queues so the tail
            # isn't gated by the slower SWDGE drain
            half = QE // 2
            if s == NS - 1 and g >= 2 * CS - 2:
                qs = (nc.sync, nc.scalar)
            else:
                qs = [(nc.sync, nc.gpsimd), (nc.gpsimd, nc.sync)][g % 2]
            for z, qeng in enumerate(qs):
                qeng.dma_start(
                    out=out[128 * ti:128 * (ti + 1),
                            QE * eoq + half * z:QE * eoq + half * (z + 1)],
                    in_=fo[:, half * z:half * (z + 1)],
                )

        # ================= program order =================
        # strip 0 inputs
        xtoks = {}
        for ti in range(TC):
            xtoks[ti] = stage_x(ti)
            if ti < CS:
                transpose_x(ti, xtoks[ti])
        for ti in range(CS):
            nc.gpsimd.memset(vaug[ti], 1.0)

        qT0 = [None] * EC

        def q_evict(eo, ps):
            o = qTp.tile([128, QT], BF16, name="qT", tag="qT")
            nc.scalar.copy(out=o, in_=ps)
            qT0[eo] = o

        def k_evict(eo, ps):
            nc.scalar.copy(out=kT[eo][:, 0:QT], in_=ps)

        proj_half_paced(wq, 0, 0, q_evict)
        proj_half_paced(wq, 0, 4, q_evict)
        proj_half_paced(wk, 0, 0, k_evict)
        proj_half_paced(wk, 0, 4, k_evict)
        for ti in range(CS):
            for eoq in range(NE):
                v_group(ti, eoq)
        wo = load_w(Wo)
        w1 = load_w(W1)
        for ti in range(CS, TC):
            nc.gpsimd.memset(vaug[ti], 1.0)
        for ti in range(CS, TC):
            transpose_x(ti, xtoks[ti])
        projT = [projp.tile([128, TT], BF16, name="projT", tag="projT")
                 for _ in range(EC)]

        # attention strip 0 with strip-1 Q/K/V interleaved between pairs
        qT1 = [None] * EC

        def qkv1_filler(pi):
            qT1[pi] = q_group(1, pi, on_vector=True)
            k_group(1, pi, on_vector=True)
            v_group(CS + pi // NE, pi % NE, on_vector=True)

        attention_strip(0, qT0, qkv1_filler)

        # attention strip 1 with strip-0 out-proj/FFN interleaved
        def dense_filler(pi):
            if pi < 4:
                outproj_eo(wo, 0, 2 * pi)
                outproj_eo(wo, 0, 2 * pi + 1)
            else:
                ffn_group(w1, 0, 2 * (pi - 4))
                ffn_group(w1, 0, 2 * (pi - 4) + 1)

        attention_strip(1, qT1, dense_filler)

        for eo in range(EC):
            outproj_eo(wo, 1, eo)
        for g in range(2 * CS):
            ffn_group(w1, 1, g)

    nc.finalize()
    return nc


_NC_CACHE = {}


def _get_nc(shape_key):
    if shape_key not in _NC_CACHE:
        _NC_CACHE[shape_key] = build_nc(*shape_key)
    return _NC_CACHE[shape_key]


def kernel(x, Wq, Wk, Wv, Wo, bo, W1, b1):
    x = np.ascontiguousarray(np.asarray(x, dtype=np.float32))
    ws = {
        "Wq": np.ascontiguousarray(np.asarray(Wq, dtype=np.float32)),
        "Wk": np.ascontiguousarray(np.asarray(Wk, dtype=np.float32)),
        "Wv": np.ascontiguousarray(np.asarray(Wv, dtype=np.float32)),
        "Wo": np.ascontiguousarray(np.asarray(Wo, dtype=np.float32)),
        "bo": np.ascontiguousarray(np.asarray(bo, dtype=np.float32)),
        "W1": np.ascontiguousarray(np.asarray(W1, dtype=np.float32)),
        "b1": np.ascontiguousarray(np.asarray(b1, dtype=np.float32)),
    }
    B, TT, EE = x.shape
    assert B == N_CORES
    nc = _get_nc((TT, EE, H, DH))
    in_maps = [dict(ws, x=x[b]) for b in range(B)]
    res = run_bass_kernel_spmd(nc, in_maps, core_ids=list(range(N_CORES)))
    return np.stack([res.results[b]["out"] for b in range(B)], axis=0).astype(
        np.float32
    )
